# revision 1
# baseline (speedup 1.0000x reference)
"""Trainium2 Bass kernel for nn_CrossLayer (dense transformer layer).

Sharding: sequence-parallel over 8 cores (2 samples x 4 token-chunks of 512).
Each core computes its 512 token rows through CA -> SA -> FFN. K/V for all 16
heads are computed from each core's own rows and AllGather'd (bf16) across the
4 cores of its sample, once per attention block.

On-chip layout: activations feature-major [dim(128p x 8c), tok] so every
matmul contracts over partitions. RMSNorm partition-sums via ones-matmuls on
PE; RoPE rotate-half via a constant +-1 block matrix on PE; softmax
denominators via an appended ones column on V; exp without max subtraction
(scores are O(1): q/k are rms-normalized and /sqrt(d)).
"""

import math
import sys
import types

import numpy as np
import ml_dtypes

B, N, DIM, HID, H, D = 2, 2048, 1024, 4096, 16, 64
TOK = 512  # tokens per core
NCORES = 8
EPS = 1e-6
THETA = 10000.0
P = 128
KO = DIM // P  # 8 contraction chunks
HH = H // 2  # 8 head pairs
HC = HID // P  # 32 hidden chunks
TC = TOK // P  # 4 token chunks per core
NR = 4  # ranks per replica group
VW = D + 1  # v columns + ones column

BF = ml_dtypes.bfloat16

_cache = {}


def _lhsT_layout(W):
    """[K, M] -> [M//128, 128(K%128), K//128, 128(M%128)]: SBUF slices are
    matmul lhsT tiles [128, 128]."""
    K, M = W.shape
    return (
        W.reshape(K // P, P, M // P, P).transpose(2, 1, 0, 3).astype(BF).copy()
    )


def _rhs_layout(W):
    """[K, M] -> [128, K//128, M] rhs-style."""
    K, M = W.shape
    return W.reshape(K // P, P, M).transpose(1, 0, 2).astype(BF).copy()


def _featmajor(x):
    """[tok, dim] -> [128, dim//128, tok] float32."""
    return x.T.reshape(DIM // P, P, x.shape[0]).transpose(1, 0, 2).copy()


def _rope_tables(pos):
    """pos [TOK] int32 -> cos/sin [128, TOK] (2 heads stacked) bf16."""
    invf = 1.0 / (THETA ** (np.arange(0, D, 2, dtype=np.float64) / D))  # [32]
    ang = pos.astype(np.float64)[None, :] * invf[:, None]  # [32, TOK]
    c = np.cos(ang)
    s = np.sin(ang)
    c64 = np.concatenate([c, c], axis=0)  # [64, TOK]
    s64 = np.concatenate([s, s], axis=0)
    c128 = np.concatenate([c64, c64], axis=0).astype(BF)  # [128, TOK]
    s128 = np.concatenate([s64, s64], axis=0).astype(BF)
    return c128.copy(), s128.copy()


def _install_ntff_hook():
    try:
        from trn_agent_boot.trn_boot import _ntff_profile_via_ctypes
    except ImportError:
        return
    if "antenv.axon_hooks" in sys.modules:
        return
    try:
        hook = _ntff_profile_via_ctypes("/opt/axon/libaxon_pjrt.so")
    except OSError:
        return
    mod = types.ModuleType("antenv.axon_hooks")
    mod.get_axon_ntff_profile_hook = lambda: hook
    mod.set_axon_ntff_profile_hook = lambda h: None
    sys.modules["antenv.axon_hooks"] = mod
    import antenv

    antenv.axon_hooks = mod


def _split_multiwait(nc):
    """This walrus only supports one sync-wait on CTRL-encoded instructions
    (Drain/NoOp); hoist excess waits onto single-wait NoOps placed before."""
    from concourse import mybir

    n_split = 0
    for f in nc.m.functions:
        for bb in f.blocks:
            new = []
            changed = False
            for ins in bb.instructions:
                si = ins.sync_info
                if (
                    si is not None
                    and si.on_wait is not None
                    and len(si.on_wait) > 1
                ):
                    waits = list(si.on_wait)
                    keep, rest = waits[:1], waits[1:]
                    for k, w in enumerate(rest):
                        new.append(
                            mybir.InstNoOp(
                                name=f"{ins.name}-wsplit{k}",
                                engine=ins.engine,
                                sync_info=mybir.SyncInfo(
                                    on_wait=[w], on_update=[]
                                ),
                                bass_nofuse=True,
                            )
                        )
                    si.on_wait = keep
                    n_split += 1
                    changed = True
                new.append(ins)
            if changed:
                bb.instructions = new
    return n_split


def _build_bass():
    from contextlib import ExitStack

    import concourse.bass as bass
    import concourse.tile as tile
    from concourse import mybir

    f32 = mybir.dt.float32
    bf16 = mybir.dt.bfloat16
    AF = mybir.ActivationFunctionType

    nc = bass.Bass(num_devices=NCORES)

    def inp(name, shape, dt=bf16):
        return nc.dram_tensor(name, shape, dt, kind="ExternalInput")

    tgtT = inp("tgtT", [P, KO, TOK], f32)
    srcTb = inp("srcTb", [P, KO, TOK])
    cosq = inp("cosq", [P, TOK])
    sinq = inp("sinq", [P, TOK])
    coskca = inp("coskca", [P, TOK])
    sinkca = inp("sinkca", [P, TOK])
    caWq = inp("caWq", [HH, P, KO, P])
    caWk = inp("caWk", [HH, P, KO, P])
    caWv = inp("caWv", [P, KO, DIM])
    caWo = inp("caWo", [KO, P, KO, P])
    saWq = inp("saWq", [HH, P, KO, P])
    saWk = inp("saWk", [HH, P, KO, P])
    saWv = inp("saWv", [P, KO, DIM])
    saWo = inp("saWo", [KO, P, KO, P])
    W1i = inp("W1", [HC, P, KO, P])
    W3i = inp("W3", [HC, P, KO, P])
    W2i = inp("W2", [KO, P, HC, P])
    blk2 = inp("blk2", [P, 2])  # per-head ssq lhsT (block ones)
    mq_ca = inp("mq_ca", [2, P])  # rsqrt bcast lhsT with qn folded
    mk_ca = inp("mk_ca", [2, P])
    mq_sa = inp("mq_sa", [2, P])
    mk_sa = inp("mk_sa", [2, P])
    rotm = inp("rotm", [P, P])  # rotate-half (2-head block diag) lhsT
    ones_c = inp("ones_c", [P, 1])  # y-norm ssq lhsT
    ones_r128 = inp("ones_r128", [1, P])  # y-norm bcast lhsT

    outT = nc.dram_tensor("outT", [P, KO, TOK], f32, kind="ExternalOutput")

    groups = [[0, 1, 2, 3], [4, 5, 6, 7]]
    KWORDS = P * HH * TOK  # k bf16 words per rank
    VWORDS = P * TC * H * VW  # v bf16 words per rank

    with tile.TileContext(nc) as tc:
        ctx = ExitStack()
        with ctx:
            sing = ctx.enter_context(tc.tile_pool(name="sing", bufs=1))
            wpool = ctx.enter_context(tc.tile_pool(name="wpool", bufs=2))
            w2pool = ctx.enter_context(tc.tile_pool(name="w2pool", bufs=2))
            work = ctx.enter_context(tc.tile_pool(name="work", bufs=3))
            probp = ctx.enter_context(tc.tile_pool(name="probp", bufs=2))
            stat = ctx.enter_context(tc.tile_pool(name="stat", bufs=2))
            kvpool = ctx.enter_context(tc.tile_pool(name="kvpool", bufs=1))
            dram = ctx.enter_context(
                tc.tile_pool(name="dram", bufs=1, space="DRAM")
            )
            pp = ctx.enter_context(tc.tile_pool(name="pp", bufs=2, space="PSUM"))
            ps_s = ctx.enter_context(
                tc.tile_pool(name="ps_s", bufs=2, space="PSUM")
            )
            ps_x = ctx.enter_context(
                tc.tile_pool(name="ps_x", bufs=1, space="PSUM")
            )

            # ---- resident tiles
            resid = sing.tile([P, KO, TOK], f32)
            nc.sync.dma_start(resid[:], tgtT[:])
            srcT_sb = kvpool.tile([P, KO, TOK], bf16, tag="xT", name="srcT_sb")
            nc.sync.dma_start(srcT_sb[:], srcTb[:])
            cosq_sb = sing.tile([P, TOK], bf16)
            nc.sync.dma_start(cosq_sb[:], cosq[:])
            sinq_sb = sing.tile([P, TOK], bf16)
            nc.sync.dma_start(sinq_sb[:], sinq[:])
            coskca_sb = sing.tile([P, TOK], bf16)
            nc.sync.dma_start(coskca_sb[:], coskca[:])
            sinkca_sb = sing.tile([P, TOK], bf16)
            nc.sync.dma_start(sinkca_sb[:], sinkca[:])
            blk2_sb = sing.tile([P, 2], bf16)
            nc.sync.dma_start(blk2_sb[:], blk2[:])
            masks_sb = {}
            for name, t in (
                ("mq_ca", mq_ca),
                ("mk_ca", mk_ca),
                ("mq_sa", mq_sa),
                ("mk_sa", mk_sa),
            ):
                m = sing.tile([2, P], bf16, name=name)
                nc.sync.dma_start(m[:], t[:])
                masks_sb[name] = m
            rotm_sb = sing.tile([P, P], bf16)
            nc.sync.dma_start(rotm_sb[:], rotm[:])
            ones_c_sb = sing.tile([P, 1], bf16)
            nc.sync.dma_start(ones_c_sb[:], ones_c[:])
            ones_r128_sb = sing.tile([1, P], bf16)
            nc.sync.dma_start(ones_r128_sb[:], ones_r128[:])
            eps_sb = sing.tile([2, 1], mybir.dt.float32)
            nc.vector.memset(eps_sb[:], float(EPS))

            def norm_rope_one(psum_q, mask_sb, cos_sb, sin_sb, dst):
                """psum_q [128(2 heads), TOK] f32 -> dst bf16: rms-normed,
                qn-scaled, roped."""
                raw = stat.tile([P, TOK], f32, tag="raw", name="raw")
                nc.vector.tensor_copy(raw[:], psum_q[:])
                sq = work.tile([P, TOK], bf16, tag="ysq", name="sq")
                nc.vector.tensor_mul(sq[:], raw[:], raw[:])
                ssq = pp.tile([2, TOK], f32, tag="pp", name="ssq")
                nc.tensor.matmul(ssq[:], blk2_sb[:], sq[:], start=True, stop=True)
                # rsqrt(mean+eps) = exp(-0.5*ln(mean+eps)); Ln/Exp share one
                # ACT table set (natural_log_exp) with the attention exps
                lnt = stat.tile([2, TOK], f32, tag="lnt", name="lnt")
                nc.scalar.activation(
                    lnt[:], ssq[:], AF.Ln, bias=eps_sb[:], scale=1.0 / D
                )
                rs = stat.tile([2, TOK], bf16, tag="rs", name="rs")
                nc.scalar.activation(rs[:], lnt[:], AF.Exp, scale=-0.5)
                bc = pp.tile([P, TOK], f32, tag="pp", name="bc")
                nc.tensor.matmul(bc[:], mask_sb[:], rs[:], start=True, stop=True)
                v1 = stat.tile([P, TOK], bf16, tag="v1", name="v1")
                nc.vector.tensor_mul(v1[:], raw[:], bc[:])
                rot_ps = pp.tile([P, TOK], f32, tag="pp", name="rot_ps")
                nc.tensor.matmul(
                    rot_ps[:], rotm_sb[:], v1[:], start=True, stop=True
                )
                rot = stat.tile([P, TOK], bf16, tag="rot", name="rot")
                nc.scalar.copy(rot[:], rot_ps[:])
                t1 = stat.tile([P, TOK], bf16, tag="t1", name="t1")
                nc.vector.tensor_mul(t1[:], v1[:], cos_sb[:])
                nc.vector.tensor_mul(dst, rot[:], sin_sb[:])
                nc.vector.tensor_add(dst, t1[:], dst)

            def rmsnorm_feat(src_f32, dst_bf16):
                """Feature-major RMSNorm: dst = src * rsqrt(mean(src^2))."""
                ssq = pp.tile([1, TOK], f32, tag="pp", name="yssq")
                for c in range(KO):
                    sq = work.tile([P, TOK], bf16, tag="ysq", name="ynsq")
                    nc.vector.tensor_mul(sq[:], src_f32[:, c], src_f32[:, c])
                    nc.tensor.matmul(
                        ssq[:],
                        ones_c_sb[:],
                        sq[:],
                        start=(c == 0),
                        stop=(c == KO - 1),
                    )
                lnt = stat.tile([1, TOK], f32, tag="lnt", name="ylnt")
                nc.scalar.activation(
                    lnt[:], ssq[:], AF.Ln, bias=eps_sb[:1], scale=1.0 / DIM
                )
                rs = stat.tile([1, TOK], bf16, tag="rs", name="yrs")
                nc.scalar.activation(rs[:], lnt[:], AF.Exp, scale=-0.5)
                bc = pp.tile([P, TOK], f32, tag="pp", name="ybc")
                nc.tensor.matmul(
                    bc[:], ones_r128_sb[:], rs[:], start=True, stop=True
                )
                for c in range(KO):
                    nc.vector.tensor_mul(dst_bf16[:, c], src_f32[:, c], bc[:])

            def attention_block(y_sb, kvsrc_sb, Wq_t, Wk_t, Wv_t, Wo_t,
                                mq, mk, cosk, sink):
                """One attention block; y_sb bf16 [P,KO,TOK] is the q-side
                input, kvsrc_sb the kv-side input. Adds Wo output into resid."""
                # --- k projection + norm/rope from my rows
                k_mine = kvpool.tile([P, HH, TOK], bf16, tag="kq", name="k_mine")
                for g in range(2):  # stream Wk in halves
                    wk = wpool.tile([P, 4, KO, P], bf16, tag="w1m", name="wk")
                    nc.sync.dma_start(
                        wk[:],
                        Wk_t[g * 4 : (g + 1) * 4].rearrange(
                            "g p ko m -> p g ko m"
                        ),
                    )
                    for j in range(4):
                        hh = g * 4 + j
                        pk = pp.tile([P, TOK], f32, tag="pp", name="pk")
                        for c in range(KO):
                            nc.tensor.matmul(
                                pk[:],
                                wk[:, j, c],
                                kvsrc_sb[:, c],
                                start=(c == 0),
                                stop=(c == KO - 1),
                            )
                        norm_rope_one(pk, mk, cosk, sink, k_mine[:, hh])

                # --- v projection (token-major, with ones column)
                v_mine = kvpool.tile(
                    [P, TC, H, VW], bf16, tag="vm", name="v_mine"
                )
                nc.vector.memset(v_mine[:, :, :, D : D + 1], 1.0)
                for nh in range(2):
                    wv = wpool.tile([P, KO, TOK], bf16, tag="w1m", name="wv")
                    nc.sync.dma_start(
                        wv[:], Wv_t[:, :, nh * TOK : (nh + 1) * TOK]
                    )
                    for t4 in range(TC):
                        pv = pp.tile([P, TOK], f32, tag="pp", name="pv")
                        for c in range(KO):
                            nc.tensor.matmul(
                                pv[:],
                                kvsrc_sb[:, c, t4 * P : (t4 + 1) * P],
                                wv[:, c],
                                start=(c == 0),
                                stop=(c == KO - 1),
                            )
                        nc.vector.tensor_copy(
                            v_mine[:, t4, nh * 8 : (nh + 1) * 8, 0:D],
                            pv[:].rearrange("p (h d) -> p h d", d=D),
                        )

                # --- allgather k/v across my sample's 4 cores
                kv_in = dram.tile([KWORDS + VWORDS], bf16, tag="kv_in")
                nc.sync.dma_start(
                    kv_in[:KWORDS].rearrange(
                        "(p h t) -> p h t", p=P, h=HH, t=TOK
                    ),
                    k_mine[:],
                )
                nc.sync.dma_start(
                    kv_in[KWORDS:].rearrange(
                        "(p a b c) -> p a b c", p=P, a=TC, b=H, c=VW
                    ),
                    v_mine[:],
                )
                kv_out = dram.tile([NR, KWORDS + VWORDS], bf16, tag="kv_out")
                nc.gpsimd.collective_compute(
                    "AllGather",
                    mybir.AluOpType.bypass,
                    replica_groups=groups,
                    ins=[kv_in.opt()],
                    outs=[kv_out.opt()],
                )
                k_full = kvpool.tile(
                    [P, HH, NR, TOK], bf16, tag="k_full", name="k_full"
                )
                v_full = kvpool.tile(
                    [P, NR, TC, H, VW], bf16, tag="v_full", name="v_full"
                )
                for r in range(NR):
                    nc.sync.dma_start(
                        k_full[:, :, r],
                        kv_out[r, :KWORDS].rearrange(
                            "(p h t) -> p h t", p=P, h=HH, t=TOK
                        ),
                    )
                    nc.sync.dma_start(
                        v_full[:, r],
                        kv_out[r, KWORDS:].rearrange(
                            "(p a b c) -> p a b c", p=P, a=TC, b=H, c=VW
                        ),
                    )

                # --- q projection + norm + rope (overlaps the collective)
                q_sb = kvpool.tile([P, HH, TOK], bf16, tag="kq", name="q_sb")
                for g in range(2):
                    wq = wpool.tile([P, 4, KO, P], bf16, tag="w1m", name="wq")
                    nc.sync.dma_start(
                        wq[:],
                        Wq_t[g * 4 : (g + 1) * 4].rearrange(
                            "g p ko m -> p g ko m"
                        ),
                    )
                    for j in range(4):
                        hh = g * 4 + j
                        pq = pp.tile([P, TOK], f32, tag="pp", name="pq")
                        for c in range(KO):
                            nc.tensor.matmul(
                                pq[:],
                                wq[:, j, c],
                                y_sb[:, c],
                                start=(c == 0),
                                stop=(c == KO - 1),
                            )
                        norm_rope_one(pq, mq, cosq_sb, sinq_sb, q_sb[:, hh])

                # --- attention: 2 heads share one exp; denominators ride in
                # row 64 of the px accumulators (ones column of v)
                xT = kvpool.tile([P, HH, TOK], bf16, tag="xT", name="xT")
                dens = kvpool.tile([D + 1, H, TOK], bf16, tag="dens", name="dens")
                for hh in range(HH):
                    px = [
                        ps_x.tile([VW, TOK], f32, tag=f"px{i}", name=f"px{i}")
                        for i in range(2)
                    ]
                    for kc in range(H):  # 16 k-chunks of 128 tokens
                        r, tcl = kc // TC, kc % TC
                        ps = ps_s.tile([P, 2 * TOK], f32, tag="ps", name="ps")
                        for i in range(2):
                            off = i * D
                            nc.tensor.matmul(
                                ps[:, i * TOK : (i + 1) * TOK],
                                k_full[
                                    off : off + D,
                                    hh,
                                    r,
                                    tcl * P : (tcl + 1) * P,
                                ],
                                q_sb[off : off + D, hh],
                                start=True,
                                stop=True,
                            )
                        prob = probp.tile(
                            [P, 2 * TOK], bf16, tag="prob", name="prob"
                        )
                        nc.scalar.activation(
                            prob[:], ps[:], AF.Exp, scale=1.0 / math.sqrt(D)
                        )
                        for i in range(2):
                            h = hh * 2 + i
                            nc.tensor.matmul(
                                px[i][:],
                                v_full[:, r, tcl, h],
                                prob[:, i * TOK : (i + 1) * TOK],
                                start=(kc == 0),
                                stop=(kc == H - 1),
                            )
                    for i in range(2):
                        h = hh * 2 + i
                        # denom row lives on partition 64; keep it there
                        nc.vector.tensor_copy(
                            dens[D : D + 1, h], px[i][D : D + 1]
                        )
                        # 64-channel copy may retarget the other half-window
                        nc.vector.tensor_copy(
                            xT[i * D : (i + 1) * D, hh], px[i][0:D]
                        )

                # --- softmax denominators: one reciprocal, broadcast via DRAM
                dflat = dens[D : D + 1].rearrange("o h t -> o (h t)")
                nc.scalar.activation(dflat, dflat, AF.Ln)
                nc.scalar.activation(dflat, dflat, AF.Exp, scale=-1.0)
                db = dram.tile([H * TOK], bf16, tag="db")
                nc.sync.dma_start(
                    db[:].rearrange("(o h t) -> o h t", o=1, h=H),
                    dens[D : D + 1],
                )
                rec_bc = kvpool.tile(
                    [P, HH, TOK], bf16, tag="vm", name="rec_bc"
                )
                for i in range(2):
                    src = bass.AP(
                        tensor=db.tensor,
                        offset=db.offset + i * TOK,
                        ap=[[0, D], [2 * TOK, HH], [1, TOK]],
                    )
                    nc.sync.dma_start(rec_bc[i * D : (i + 1) * D], src)
                for hh in range(HH):
                    nc.vector.tensor_mul(
                        xT[:, hh], xT[:, hh], rec_bc[:, hh]
                    )

                # --- Wo projection, accumulate into resid
                for g in range(2):
                    wo = wpool.tile([P, 4, KO, P], bf16, tag="w1m", name="wo")
                    nc.sync.dma_start(
                        wo[:],
                        Wo_t[g * 4 : (g + 1) * 4].rearrange(
                            "g p ko m -> p g ko m"
                        ),
                    )
                    for j in range(4):
                        oc = g * 4 + j
                        po = pp.tile([P, TOK], f32, tag="pp", name="po")
                        for c in range(KO):
                            nc.tensor.matmul(
                                po[:],
                                wo[:, j, c],
                                xT[:, c],
                                start=(c == 0),
                                stop=(c == KO - 1),
                            )
                        nc.vector.tensor_add(resid[:, oc], resid[:, oc], po[:])

            # ================= cross-attention =================
            yT = sing.tile([P, KO, TOK], bf16, name="yT")
            rmsnorm_feat(resid, yT)
            attention_block(
                yT, srcT_sb, caWq, caWk, caWv, caWo,
                masks_sb["mq_ca"], masks_sb["mk_ca"], coskca_sb, sinkca_sb,
            )

            # ================= self-attention =================
            rmsnorm_feat(resid, yT)
            attention_block(
                yT, yT, saWq, saWk, saWv, saWo,
                masks_sb["mq_sa"], masks_sb["mk_sa"], cosq_sb, sinq_sb,
            )

            # ================= FFN =================
            rmsnorm_feat(resid, yT)
            hT = kvpool.tile([P, HC, TOK], bf16, tag="k_full", name="hT")
            for g in range(8):  # stream W1/W3 in eighths
                w1 = wpool.tile([P, 4, KO, P], bf16, tag="w1m", name="w1")
                nc.sync.dma_start(
                    w1[:],
                    W1i[g * 4 : (g + 1) * 4].rearrange("g p ko m -> p g ko m"),
                )
                w3 = wpool.tile([P, 4, KO, P], bf16, tag="w1m", name="w3")
                nc.sync.dma_start(
                    w3[:],
                    W3i[g * 4 : (g + 1) * 4].rearrange("g p ko m -> p g ko m"),
                )
                for j in range(4):
                    hc = g * 4 + j
                    p1 = pp.tile([P, TOK], f32, tag="pp", name="p1")
                    for c in range(KO):
                        nc.tensor.matmul(
                            p1[:], w1[:, j, c], yT[:, c],
                            start=(c == 0), stop=(c == KO - 1),
                        )
                    p3 = pp.tile([P, TOK], f32, tag="pp", name="p3")
                    for c in range(KO):
                        nc.tensor.matmul(
                            p3[:], w3[:, j, c], yT[:, c],
                            start=(c == 0), stop=(c == KO - 1),
                        )
                    s1 = stat.tile([P, TOK], f32, tag="raw", name="s1")
                    nc.scalar.activation(s1[:], p1[:], AF.Silu)
                    nc.vector.tensor_mul(hT[:, hc], s1[:], p3[:])
            for oc in range(KO):
                w2 = w2pool.tile([P, HC, P], bf16, tag="w2", name="w2")
                nc.sync.dma_start(w2[:], W2i[oc])
                po = pp.tile([P, TOK], f32, tag="pp", name="po2")
                for hc in range(HC):
                    nc.tensor.matmul(
                        po[:], w2[:, hc], hT[:, hc],
                        start=(hc == 0), stop=(hc == HC - 1),
                    )
                nc.vector.tensor_add(resid[:, oc], resid[:, oc], po[:])

            nc.sync.dma_start(outT[:], resid[:])

    _split_multiwait(nc)
    return nc


def _prep_inputs(inputs):
    """Full problem inputs -> list of 8 per-core in_maps."""
    tgt = np.asarray(inputs["tgt"], np.float32)
    src = np.asarray(inputs["src"], np.float32)
    tgt_pos = np.asarray(inputs["tgt_pos"], np.int32)
    src_pos = np.asarray(inputs["src_pos"], np.int32)

    pre_ca_w = np.asarray(inputs["pre_ca_w"], np.float32)
    pre_sa_w = np.asarray(inputs["pre_sa_w"], np.float32)
    pre_ffn_w = np.asarray(inputs["pre_ffn_w"], np.float32)

    def fold(Wname, w):
        return np.asarray(inputs[Wname], np.float32) * w[:, None]

    ca_Wq = fold("ca_Wq", pre_ca_w)
    ca_Wkv = np.asarray(inputs["ca_Wkv"], np.float32)
    ca_Wk, ca_Wv = ca_Wkv[:, :DIM], ca_Wkv[:, DIM:]
    ca_Wo = np.asarray(inputs["ca_Wo"], np.float32)
    sa_Wq = fold("sa_Wq", pre_sa_w)
    sa_Wkv = fold("sa_Wkv", pre_sa_w)
    sa_Wk, sa_Wv = sa_Wkv[:, :DIM], sa_Wkv[:, DIM:]
    sa_Wo = np.asarray(inputs["sa_Wo"], np.float32)
    W1 = fold("W1", pre_ffn_w)
    W3 = fold("W3", pre_ffn_w)
    W2 = np.asarray(inputs["W2"], np.float32)

    shared = {
        "caWq": _lhsT_layout(ca_Wq),
        "caWk": _lhsT_layout(ca_Wk),
        "caWv": _rhs_layout(ca_Wv),
        "caWo": _lhsT_layout(ca_Wo),
        "saWq": _lhsT_layout(sa_Wq),
        "saWk": _lhsT_layout(sa_Wk),
        "saWv": _rhs_layout(sa_Wv),
        "saWo": _lhsT_layout(sa_Wo),
        "W1": _lhsT_layout(W1),
        "W3": _lhsT_layout(W3),
        "W2": _lhsT_layout(W2),
    }

    blk2 = np.zeros((P, 2), BF)
    blk2[:D, 0] = 1
    blk2[D:, 1] = 1
    shared["blk2"] = blk2

    def head_mask(w):  # [2, 128] with per-head norm weight
        m = np.zeros((2, P), np.float32)
        m[0, :D] = w
        m[1, D:] = w
        return m.astype(BF).copy()

    shared["mq_ca"] = head_mask(np.asarray(inputs["ca_qn"], np.float32))
    shared["mk_ca"] = head_mask(np.asarray(inputs["ca_kn"], np.float32))
    shared["mq_sa"] = head_mask(np.asarray(inputs["sa_qn"], np.float32))
    shared["mk_sa"] = head_mask(np.asarray(inputs["sa_kn"], np.float32))

    r64 = np.zeros((D, D), np.float32)
    half = D // 2
    for j in range(half):
        r64[j, j + half] = -1.0  # rot[j] = -x[j+32]
        r64[j + half, j] = 1.0  # rot[j+32] = x[j]
    rt = r64.T  # lhsT (matmul computes lhsT.T @ rhs)
    rotm = np.zeros((P, P), np.float32)
    rotm[:D, :D] = rt
    rotm[D:, D:] = rt
    shared["rotm"] = rotm.astype(BF).copy()

    shared["ones_c"] = np.ones((P, 1), BF)
    shared["ones_r128"] = np.ones((1, P), BF)

    in_maps = []
    for c in range(NCORES):
        s, part = c // NR, c % NR
        rows = slice(part * TOK, (part + 1) * TOK)
        m = dict(shared)
        m["tgtT"] = _featmajor(tgt[s, rows])
        m["srcTb"] = _featmajor(src[s, rows]).astype(BF)
        cq, sq_ = _rope_tables(tgt_pos[s, rows])
        ck, sk = _rope_tables(src_pos[s, rows])
        m["cosq"], m["sinq"] = cq, sq_
        m["coskca"], m["sinkca"] = ck, sk
        in_maps.append(m)
    return in_maps


def _get_nc():
    if "nc" not in _cache:
        _cache["nc"] = _build_bass()
    return _cache["nc"]


def run(inputs, trace=False):
    """Run on 8 cores; returns (full_output, exec_time_ns_or_None)."""
    if trace:
        _install_ntff_hook()
    from concourse.bass_utils import run_bass_kernel_spmd

    in_maps = _prep_inputs(inputs)
    nc = _get_nc()
    res = run_bass_kernel_spmd(
        nc, in_maps, core_ids=list(range(NCORES)), trace=trace
    )
    out = np.empty((B, N, DIM), np.float32)
    for c in range(NCORES):
        s, part = c // NR, c % NR
        arr = np.asarray(res.results[c]["outT"])  # [128, 8, TOK]
        rows = slice(part * TOK, (part + 1) * TOK)
        out[s, rows] = np.transpose(arr, (2, 1, 0)).reshape(TOK, DIM)
    return out, res.exec_time_ns


def kernel(**inputs):
    out, _ = run(inputs, trace=False)
    return out



# revision 9
# speedup vs baseline: 1.1673x; 1.1673x over previous
"""Trainium2 Bass kernel for nn_CrossLayer (dense transformer layer).

Sharding: sequence-parallel over 8 cores (2 samples x 4 token-chunks of 512).
Each core computes its 512 token rows through CA -> SA -> FFN. K/V for all 16
heads are computed from each core's own rows (fp8) and AllGather'd across the
4 cores of its sample as two collectives (k first, then v) launched as early
as possible; q-projection and score/exp run-ahead hide the transfer.

On-chip layout: activations feature-major [dim(128p x 8c), tok] so every
matmul contracts over partitions. RMSNorm partition-sums via ones-matmuls on
PE; RoPE rotate-half via a constant +-1 block matrix on PE; softmax
denominators ride in row 64 of the av accumulators (ones column of v) and are
reciprocal'd on DVE + broadcast through a tiny DRAM bounce per head-pair,
overlapped with the next head's scores. exp uses bias=-2 so fp8/bf16 prob
tiles stay in range (cancels in the normalization).
"""

import math
import sys
import types

import numpy as np
import ml_dtypes

B, N, DIM, HID, H, D = 2, 2048, 1024, 4096, 16, 64
TOK = 512  # tokens per core
NCORES = 8
EPS = 1e-6
THETA = 10000.0
P = 128
KO = DIM // P  # 8 contraction chunks
HH = H // 2  # 8 head pairs
HC = HID // P  # 32 hidden chunks
TC = TOK // P  # 4 token chunks per core
NR = 4  # ranks per replica group
VW = D + 1  # v columns + ones column

BF = ml_dtypes.bfloat16

_cache = {}


def _lhsT_layout(W):
    """[K, M] -> [M//128, 128(K%128), K//128, 128(M%128)]: SBUF slices are
    matmul lhsT tiles [128, 128]."""
    K, M = W.shape
    return (
        W.reshape(K // P, P, M // P, P).transpose(2, 1, 0, 3).astype(BF).copy()
    )


def _rhs_layout(W):
    """[K, M] -> [128, K//128, M] rhs-style."""
    K, M = W.shape
    return W.reshape(K // P, P, M).transpose(1, 0, 2).astype(BF).copy()


def _featmajor(x):
    """[tok, dim] -> [128, dim//128, tok] float32."""
    return x.T.reshape(DIM // P, P, x.shape[0]).transpose(1, 0, 2).copy()


def _rope_tables(pos):
    """pos [TOK] int32 -> cos/sin [128, TOK] (2 heads stacked) bf16."""
    invf = 1.0 / (THETA ** (np.arange(0, D, 2, dtype=np.float64) / D))  # [32]
    ang = pos.astype(np.float64)[None, :] * invf[:, None]  # [32, TOK]
    c = np.cos(ang)
    s = np.sin(ang)
    c64 = np.concatenate([c, c], axis=0)  # [64, TOK]
    s64 = np.concatenate([s, s], axis=0)
    c128 = np.concatenate([c64, c64], axis=0).astype(BF)  # [128, TOK]
    s128 = np.concatenate([s64, s64], axis=0).astype(BF)
    return c128.copy(), s128.copy()


def _install_ntff_hook():
    try:
        from trn_agent_boot.trn_boot import _ntff_profile_via_ctypes
    except ImportError:
        return
    if "antenv.axon_hooks" in sys.modules:
        return
    try:
        hook = _ntff_profile_via_ctypes("/opt/axon/libaxon_pjrt.so")
    except OSError:
        return
    mod = types.ModuleType("antenv.axon_hooks")
    mod.get_axon_ntff_profile_hook = lambda: hook
    mod.set_axon_ntff_profile_hook = lambda h: None
    sys.modules["antenv.axon_hooks"] = mod
    import antenv

    antenv.axon_hooks = mod


def _split_multiwait(nc):
    """This walrus only supports one sync-wait on CTRL-encoded instructions
    (Drain/NoOp); hoist excess waits onto single-wait NoOps placed before."""
    from concourse import mybir

    n_split = 0
    for f in nc.m.functions:
        for bb in f.blocks:
            new = []
            changed = False
            for ins in bb.instructions:
                si = ins.sync_info
                if (
                    si is not None
                    and si.on_wait is not None
                    and len(si.on_wait) > 1
                ):
                    waits = list(si.on_wait)
                    keep, rest = waits[:1], waits[1:]
                    for k, w in enumerate(rest):
                        new.append(
                            mybir.InstNoOp(
                                name=f"{ins.name}-wsplit{k}",
                                engine=ins.engine,
                                sync_info=mybir.SyncInfo(
                                    on_wait=[w], on_update=[]
                                ),
                                bass_nofuse=True,
                            )
                        )
                    si.on_wait = keep
                    n_split += 1
                    changed = True
                new.append(ins)
            if changed:
                bb.instructions = new
    return n_split


def _build_bass():
    from contextlib import ExitStack

    import concourse.bass as bass
    import concourse.tile as tile
    from concourse import mybir

    f32 = mybir.dt.float32
    bf16 = mybir.dt.bfloat16
    f8 = mybir.dt.float8e4
    AF = mybir.ActivationFunctionType

    nc = bass.Bass(num_devices=NCORES)

    def inp(name, shape, dt=bf16):
        return nc.dram_tensor(name, shape, dt, kind="ExternalInput")

    tgtT = inp("tgtT", [P, KO, TOK], f32)
    srcTb = inp("srcTb", [P, KO, TOK])
    cosq = inp("cosq", [P, TOK])
    sinq = inp("sinq", [P, TOK])
    coskca = inp("coskca", [P, TOK])
    sinkca = inp("sinkca", [P, TOK])
    caWq = inp("caWq", [HH, P, KO, P])
    caWk = inp("caWk", [HH, P, KO, P])
    caWv = inp("caWv", [P, KO, DIM])
    caWo = inp("caWo", [KO, P, KO, P])
    saWq = inp("saWq", [HH, P, KO, P])
    saWk = inp("saWk", [HH, P, KO, P])
    saWv = inp("saWv", [P, KO, DIM])
    saWo = inp("saWo", [KO, P, KO, P])
    W1i = inp("W1", [HC, P, KO, P])
    W3i = inp("W3", [HC, P, KO, P])
    W2i = inp("W2", [KO, P, HC, P])
    blk2 = inp("blk2", [P, 2])  # per-head ssq lhsT (block ones)
    mq_ca = inp("mq_ca", [2, P])  # rsqrt bcast lhsT with qn folded
    mk_ca = inp("mk_ca", [2, P])
    mq_sa = inp("mq_sa", [2, P])
    mk_sa = inp("mk_sa", [2, P])
    rotm = inp("rotm", [P, P])  # rotate-half (2-head block diag) lhsT
    ones_c = inp("ones_c", [P, 1])  # y-norm ssq lhsT
    ones_r128 = inp("ones_r128", [1, P])  # y-norm bcast lhsT

    outT = nc.dram_tensor("outT", [P, KO, TOK], f32, kind="ExternalOutput")

    groups = [[0, 1, 2, 3], [4, 5, 6, 7]]
    KWORDS = P * HH * TOK  # k fp8 bytes per rank
    VWORDS = P * TC * H * VW  # v fp8 bytes per rank

    with tile.TileContext(nc) as tc:
        ctx = ExitStack()
        with ctx, nc.allow_low_precision("bf16/fp8 intermediates by design"):
            sing = ctx.enter_context(tc.tile_pool(name="sing", bufs=1))
            wpool = ctx.enter_context(tc.tile_pool(name="wpool", bufs=3))
            w2pool = ctx.enter_context(tc.tile_pool(name="w2pool", bufs=2))
            work = ctx.enter_context(tc.tile_pool(name="work", bufs=3))
            stat = ctx.enter_context(tc.tile_pool(name="stat", bufs=3))
            probp = ctx.enter_context(tc.tile_pool(name="probp", bufs=20))
            rpool = ctx.enter_context(tc.tile_pool(name="rpool", bufs=2))
            drpool = ctx.enter_context(tc.tile_pool(name="drpool", bufs=2))
            kvpool = ctx.enter_context(tc.tile_pool(name="kvpool", bufs=1))
            dram = ctx.enter_context(
                tc.tile_pool(name="dram", bufs=1, space="DRAM")
            )
            dbp = ctx.enter_context(
                tc.tile_pool(name="dbp", bufs=2, space="DRAM")
            )
            pp = ctx.enter_context(tc.tile_pool(name="pp", bufs=2, space="PSUM"))
            ps_s = ctx.enter_context(
                tc.tile_pool(name="ps_s", bufs=2, space="PSUM")
            )
            ps_x = ctx.enter_context(
                tc.tile_pool(name="ps_x", bufs=1, space="PSUM")
            )

            # ---- resident tiles
            srcT_sb = kvpool.tile([P, KO, TOK], bf16, tag="xT", name="srcT_sb")
            nc.sync.dma_start(srcT_sb[:], srcTb[:])
            resid = sing.tile([P, KO, TOK], f32)
            nc.sync.dma_start(resid[:], tgtT[:])
            cosq_sb = sing.tile([P, TOK], bf16)
            nc.sync.dma_start(cosq_sb[:], cosq[:])
            sinq_sb = sing.tile([P, TOK], bf16)
            nc.sync.dma_start(sinq_sb[:], sinq[:])
            coskca_sb = sing.tile([P, TOK], bf16)
            nc.sync.dma_start(coskca_sb[:], coskca[:])
            sinkca_sb = sing.tile([P, TOK], bf16)
            nc.sync.dma_start(sinkca_sb[:], sinkca[:])
            blk2_sb = sing.tile([P, 2], bf16)
            nc.sync.dma_start(blk2_sb[:], blk2[:])
            masks_sb = {}
            for name, t in (
                ("mq_ca", mq_ca),
                ("mk_ca", mk_ca),
                ("mq_sa", mq_sa),
                ("mk_sa", mk_sa),
            ):
                m = sing.tile([2, P], bf16, name=name)
                nc.sync.dma_start(m[:], t[:])
                masks_sb[name] = m
            rotm_sb = sing.tile([P, P], bf16)
            nc.sync.dma_start(rotm_sb[:], rotm[:])
            ones_c_sb = sing.tile([P, 1], bf16)
            nc.sync.dma_start(ones_c_sb[:], ones_c[:])
            ones_r128_sb = sing.tile([1, P], bf16)
            nc.sync.dma_start(ones_r128_sb[:], ones_r128[:])
            eps_sb = sing.tile([2, 1], mybir.dt.float32)
            nc.vector.memset(eps_sb[:], float(EPS))
            negb_sb = sing.tile([P, 1], mybir.dt.float32)
            nc.vector.memset(negb_sb[:], -2.0)

            def norm_rope_one(psum_q, mask_sb, cos_sb, sin_sb, dst):
                """psum_q [128(2 heads), TOK] f32 -> dst fp8: rms-normed,
                qn-scaled, roped."""
                raw = stat.tile([P, TOK], bf16, tag="raw", name="raw")
                nc.vector.tensor_copy(raw[:], psum_q[:])
                sq = work.tile([P, TOK], bf16, tag="ysq", name="sq")
                nc.vector.tensor_mul(sq[:], raw[:], raw[:])
                ssq = pp.tile([2, TOK], f32, tag="pp", name="ssq")
                nc.tensor.matmul(ssq[:], blk2_sb[:], sq[:], start=True, stop=True)
                # rsqrt(mean+eps) = exp(-0.5*ln(mean+eps)); Ln/Exp share one
                # ACT table set (natural_log_exp) with the attention exps
                lnt = stat.tile([2, TOK], f32, tag="lnt", name="lnt")
                nc.scalar.activation(
                    lnt[:], ssq[:], AF.Ln, bias=eps_sb[:], scale=1.0 / D
                )
                rs = stat.tile([2, TOK], bf16, tag="rs", name="rs")
                nc.scalar.activation(rs[:], lnt[:], AF.Exp, scale=-0.5)
                bc = pp.tile([P, TOK], f32, tag="pp", name="bc")
                nc.tensor.matmul(bc[:], mask_sb[:], rs[:], start=True, stop=True)
                v1 = stat.tile([P, TOK], bf16, tag="v1", name="v1")
                nc.vector.tensor_mul(v1[:], raw[:], bc[:])
                rot_ps = pp.tile([P, TOK], f32, tag="pp", name="rot_ps")
                nc.tensor.matmul(
                    rot_ps[:], rotm_sb[:], v1[:], start=True, stop=True
                )
                t1 = stat.tile([P, TOK], bf16, tag="t1", name="t1")
                nc.vector.tensor_mul(t1[:], v1[:], cos_sb[:])
                t2 = stat.tile([P, TOK], bf16, tag="t2", name="t2")
                nc.vector.tensor_mul(t2[:], rot_ps[:], sin_sb[:])
                nc.vector.tensor_add(dst, t1[:], t2[:])

            def rmsnorm_feat(src_f32, dst_bf16):
                """Feature-major RMSNorm: dst = src * rsqrt(mean(src^2))."""
                ssq = pp.tile([1, TOK], f32, tag="pp", name="yssq")
                for c in range(KO):
                    sq = work.tile([P, TOK], bf16, tag="ysq", name="ynsq")
                    nc.vector.tensor_mul(sq[:], src_f32[:, c], src_f32[:, c])
                    nc.tensor.matmul(
                        ssq[:],
                        ones_c_sb[:],
                        sq[:],
                        start=(c == 0),
                        stop=(c == KO - 1),
                    )
                lnt = stat.tile([1, TOK], f32, tag="lnt", name="ylnt")
                nc.scalar.activation(
                    lnt[:], ssq[:], AF.Ln, bias=eps_sb[:1], scale=1.0 / DIM
                )
                rs = stat.tile([1, TOK], bf16, tag="rs", name="yrs")
                nc.scalar.activation(rs[:], lnt[:], AF.Exp, scale=-0.5)
                bc = pp.tile([P, TOK], f32, tag="pp", name="ybc")
                nc.tensor.matmul(
                    bc[:], ones_r128_sb[:], rs[:], start=True, stop=True
                )
                for c in range(KO):
                    nc.vector.tensor_mul(dst_bf16[:, c], src_f32[:, c], bc[:])

            def proj_heads(Wt, src_sb, dst, mask, cos_sb, sin_sb, wname):
                """Project 16 heads (8 pairs) + rmsnorm + rope into dst fp8."""
                for g in range(2):
                    w = wpool.tile([P, 4, KO, P], bf16, tag="w1m", name=wname)
                    nc.sync.dma_start(
                        w[:],
                        Wt[g * 4 : (g + 1) * 4].rearrange(
                            "g p ko m -> p g ko m"
                        ),
                    )
                    for j in range(4):
                        hh = g * 4 + j
                        pq = pp.tile([P, TOK], f32, tag="pp", name="pq")
                        for c in range(KO):
                            nc.tensor.matmul(
                                pq[:],
                                w[:, j, c],
                                src_sb[:, c],
                                start=(c == 0),
                                stop=(c == KO - 1),
                            )
                        norm_rope_one(pq, mask, cos_sb, sin_sb, dst[:, hh])

            def attention_block(kvsrc_sb, Wq_t, Wk_t, Wv_t, Wo_t,
                                mq, mk, cosk, sink, make_y, blkname):
                """One attention block. kvsrc_sb bf16 [P,KO,TOK] is the kv-side
                input; make_y() returns the q-side input (emitted after the
                collectives launch). Adds Wo output into resid."""
                # --- k projection + norm/rope from my rows (fp8), gather early
                k_mine = kvpool.tile([P, HH, TOK], f8, tag="kq", name="k_mine")
                proj_heads(Wk_t, kvsrc_sb, k_mine, mk, cosk, sink, "wk")
                kin_k = dram.tile([KWORDS], f8, tag="kin_k")
                nc.sync.dma_start(
                    kin_k[:].rearrange("(p h t) -> p h t", p=P, h=HH, t=TOK),
                    k_mine[:],
                )
                kout_k = dram.tile([NR, KWORDS], f8, tag="kout_k")
                nc.gpsimd.collective_compute(
                    "AllGather",
                    mybir.AluOpType.bypass,
                    replica_groups=groups,
                    ins=[kin_k.opt()],
                    outs=[kout_k.opt()],
                )

                # --- v projection (token-major, with ones column), fp8
                v_mine = kvpool.tile(
                    [P, TC, H, VW], f8, tag="vm", name="v_mine"
                )
                nc.vector.memset(v_mine[:, :, :, D : D + 1], 1.0)
                for nh in range(2):
                    wv = wpool.tile([P, KO, TOK], bf16, tag="w1m", name="wv")
                    nc.sync.dma_start(
                        wv[:], Wv_t[:, :, nh * TOK : (nh + 1) * TOK]
                    )
                    for t4 in range(TC):
                        pv = pp.tile([P, TOK], f32, tag="pp", name="pv")
                        for c in range(KO):
                            nc.tensor.matmul(
                                pv[:],
                                kvsrc_sb[:, c, t4 * P : (t4 + 1) * P],
                                wv[:, c],
                                start=(c == 0),
                                stop=(c == KO - 1),
                            )
                        nc.vector.tensor_copy(
                            v_mine[:, t4, nh * 8 : (nh + 1) * 8, 0:D],
                            pv[:].rearrange("p (h d) -> p h d", d=D),
                        )
                kin_v = dram.tile([VWORDS], f8, tag="kin_v")
                nc.sync.dma_start(
                    kin_v[:].rearrange(
                        "(p a b c) -> p a b c", p=P, a=TC, b=H, c=VW
                    ),
                    v_mine[:],
                )
                kout_v = dram.tile([NR, VWORDS], f8, tag="kout_v")
                nc.gpsimd.collective_compute(
                    "AllGather",
                    mybir.AluOpType.bypass,
                    replica_groups=groups,
                    ins=[kin_v.opt()],
                    outs=[kout_v.opt()],
                )

                # --- q projection + norm + rope (overlaps the collectives)
                y_sb = make_y()
                q_sb = kvpool.tile([P, HH, TOK], f8, tag="kq", name="q_sb")
                proj_heads(Wq_t, y_sb, q_sb, mq, cosq_sb, sinq_sb, "wq")

                # --- unpack gathered k/v (emitted after q DMAs so the sync
                # queue doesn't block q-side work on the collective)
                k_all = kvpool.tile(
                    [P, HH, NR, TOK], f8, tag="k_all", name="k_all"
                )
                v_all = kvpool.tile(
                    [P, NR, TC, H, VW], f8, tag="v_all", name="v_all"
                )
                for r in range(NR):
                    nc.sync.dma_start(
                        k_all[:, :, r],
                        kout_k[r].rearrange(
                            "(p h t) -> p h t", p=P, h=HH, t=TOK
                        ),
                    )
                for r in range(NR):
                    nc.sync.dma_start(
                        v_all[:, r],
                        kout_v[r].rearrange(
                            "(p a b c) -> p a b c", p=P, a=TC, b=H, c=VW
                        ),
                    )

                # --- attention: per head-pair, all 16 score/exp chunks then
                # the 32 av matmuls (prob run-ahead absorbs collective
                # latency); denominators ride in row 64 of px (ones col of v)
                xT = kvpool.tile([P, HH, TOK], bf16, tag="xT", name="xT")
                for hh in range(HH):
                    probs = []
                    for kc in range(H):  # 16 k-chunks of 128 tokens
                        r, tcl = kc // TC, kc % TC
                        ps = ps_s.tile([P, 2 * TOK], f32, tag="ps", name="ps")
                        for i in range(2):
                            off = i * D
                            nc.tensor.matmul(
                                ps[:, i * TOK : (i + 1) * TOK],
                                k_all[
                                    off : off + D,
                                    hh,
                                    r,
                                    tcl * P : (tcl + 1) * P,
                                ],
                                q_sb[off : off + D, hh],
                                start=True,
                                stop=True,
                            )
                        prob = probp.tile(
                            [P, 2 * TOK], bf16, tag="prob", name="prob"
                        )
                        # exp(s/sqrt(d) - 2): the -2 cancels against the
                        # denominator and keeps prob magnitudes small
                        nc.scalar.activation(
                            prob[:], ps[:], AF.Exp,
                            scale=1.0 / math.sqrt(D), bias=negb_sb[:],
                        )
                        probs.append((prob, r, tcl))
                    px = [
                        ps_x.tile([VW, TOK], f32, tag=f"px{i}", name=f"px{i}")
                        for i in range(2)
                    ]
                    for idx, (prob, r, tcl) in enumerate(probs):
                        for i in range(2):
                            h = hh * 2 + i
                            nc.tensor.matmul(
                                px[i][:],
                                v_all[:, r, tcl, h],
                                prob[:, i * TOK : (i + 1) * TOK],
                                start=(idx == 0),
                                stop=(idx == H - 1),
                            )
                    # --- softmax denominators for this head pair: reciprocal
                    # on DVE, broadcast via a tiny DRAM bounce; overlaps the
                    # next head-pair's scores
                    drec = drpool.tile(
                        [VW, 2, TOK], bf16, tag="drec", name="drec"
                    )
                    for i in range(2):
                        nc.vector.reciprocal(
                            drec[D : D + 1, i], px[i][D : D + 1]
                        )
                    db = dbp.tile([2 * TOK], bf16, tag="db")
                    nc.sync.dma_start(
                        db[:].rearrange("(o a t) -> o a t", o=1, a=2),
                        drec[D : D + 1],
                    )
                    rb = rpool.tile([P, TOK], bf16, tag="rb", name="rb")
                    for i in range(2):
                        src = bass.AP(
                            tensor=db.tensor,
                            offset=db.offset + i * TOK,
                            ap=[[0, D], [1, TOK]],
                        )
                        nc.sync.dma_start(rb[i * D : (i + 1) * D], src)
                        nc.vector.tensor_copy(
                            xT[i * D : (i + 1) * D, hh], px[i][0:D]
                        )
                    nc.vector.tensor_mul(xT[:, hh], xT[:, hh], rb[:])

                # --- Wo projection, accumulate into resid
                for g in range(2):
                    wo = wpool.tile([P, 4, KO, P], bf16, tag="w1m", name="wo")
                    nc.sync.dma_start(
                        wo[:],
                        Wo_t[g * 4 : (g + 1) * 4].rearrange(
                            "g p ko m -> p g ko m"
                        ),
                    )
                    for j in range(4):
                        oc = g * 4 + j
                        po = pp.tile([P, TOK], f32, tag="pp", name="po")
                        for c in range(KO):
                            nc.tensor.matmul(
                                po[:],
                                wo[:, j, c],
                                xT[:, c],
                                start=(c == 0),
                                stop=(c == KO - 1),
                            )
                        nc.vector.tensor_add(resid[:, oc], resid[:, oc], po[:])

            # ================= cross-attention =================
            yT = sing.tile([P, KO, TOK], bf16, name="yT")

            def make_y_ca():
                rmsnorm_feat(resid, yT)
                return yT

            attention_block(
                srcT_sb, caWq, caWk, caWv, caWo,
                masks_sb["mq_ca"], masks_sb["mk_ca"], coskca_sb, sinkca_sb,
                make_y_ca, "ca",
            )

            # ================= self-attention =================
            rmsnorm_feat(resid, yT)
            attention_block(
                yT, saWq, saWk, saWv, saWo,
                masks_sb["mq_sa"], masks_sb["mk_sa"], cosq_sb, sinq_sb,
                lambda: yT, "sa",
            )

            # ================= FFN =================
            rmsnorm_feat(resid, yT)
            hT = kvpool.tile([P, HC, TOK], bf16, tag="k_all", name="hT")
            for g in range(8):  # stream W1/W3 in eighths
                w1 = wpool.tile([P, 4, KO, P], bf16, tag="w1m", name="w1")
                nc.sync.dma_start(
                    w1[:],
                    W1i[g * 4 : (g + 1) * 4].rearrange("g p ko m -> p g ko m"),
                )
                w3 = wpool.tile([P, 4, KO, P], bf16, tag="w1m", name="w3")
                nc.sync.dma_start(
                    w3[:],
                    W3i[g * 4 : (g + 1) * 4].rearrange("g p ko m -> p g ko m"),
                )
                for j in range(4):
                    hc = g * 4 + j
                    p1 = pp.tile([P, TOK], f32, tag="pp", name="p1")
                    for c in range(KO):
                        nc.tensor.matmul(
                            p1[:], w1[:, j, c], yT[:, c],
                            start=(c == 0), stop=(c == KO - 1),
                        )
                    p3 = pp.tile([P, TOK], f32, tag="pp", name="p3")
                    for c in range(KO):
                        nc.tensor.matmul(
                            p3[:], w3[:, j, c], yT[:, c],
                            start=(c == 0), stop=(c == KO - 1),
                        )
                    s1 = stat.tile([P, TOK], f32, tag="raw", name="s1")
                    nc.scalar.activation(s1[:], p1[:], AF.Silu)
                    nc.vector.tensor_mul(hT[:, hc], s1[:], p3[:])
            for oc in range(KO):
                w2 = w2pool.tile([P, HC, P], bf16, tag="w2", name="w2")
                for c4 in range(4):
                    nc.sync.dma_start(
                        w2[:, c4 * 8 : (c4 + 1) * 8],
                        W2i[oc, :, c4 * 8 : (c4 + 1) * 8],
                    )
                po = pp.tile([P, TOK], f32, tag="pp", name="po2")
                for hc in range(HC):
                    nc.tensor.matmul(
                        po[:], w2[:, hc], hT[:, hc],
                        start=(hc == 0), stop=(hc == HC - 1),
                    )
                nc.vector.tensor_add(resid[:, oc], resid[:, oc], po[:])

            nc.sync.dma_start(outT[:], resid[:])

    _split_multiwait(nc)
    return nc


def _prep_inputs(inputs):
    """Full problem inputs -> list of 8 per-core in_maps."""
    tgt = np.asarray(inputs["tgt"], np.float32)
    src = np.asarray(inputs["src"], np.float32)
    tgt_pos = np.asarray(inputs["tgt_pos"], np.int32)
    src_pos = np.asarray(inputs["src_pos"], np.int32)

    pre_ca_w = np.asarray(inputs["pre_ca_w"], np.float32)
    pre_sa_w = np.asarray(inputs["pre_sa_w"], np.float32)
    pre_ffn_w = np.asarray(inputs["pre_ffn_w"], np.float32)

    def fold(Wname, w):
        return np.asarray(inputs[Wname], np.float32) * w[:, None]

    ca_Wq = fold("ca_Wq", pre_ca_w)
    ca_Wkv = np.asarray(inputs["ca_Wkv"], np.float32)
    ca_Wk, ca_Wv = ca_Wkv[:, :DIM], ca_Wkv[:, DIM:]
    ca_Wo = np.asarray(inputs["ca_Wo"], np.float32)
    sa_Wq = fold("sa_Wq", pre_sa_w)
    sa_Wkv = fold("sa_Wkv", pre_sa_w)
    sa_Wk, sa_Wv = sa_Wkv[:, :DIM], sa_Wkv[:, DIM:]
    sa_Wo = np.asarray(inputs["sa_Wo"], np.float32)
    W1 = fold("W1", pre_ffn_w)
    W3 = fold("W3", pre_ffn_w)
    W2 = np.asarray(inputs["W2"], np.float32)

    shared = {
        "caWq": _lhsT_layout(ca_Wq),
        "caWk": _lhsT_layout(ca_Wk),
        "caWv": _rhs_layout(ca_Wv),
        "caWo": _lhsT_layout(ca_Wo),
        "saWq": _lhsT_layout(sa_Wq),
        "saWk": _lhsT_layout(sa_Wk),
        "saWv": _rhs_layout(sa_Wv),
        "saWo": _lhsT_layout(sa_Wo),
        "W1": _lhsT_layout(W1),
        "W3": _lhsT_layout(W3),
        "W2": _lhsT_layout(W2),
    }

    blk2 = np.zeros((P, 2), BF)
    blk2[:D, 0] = 1
    blk2[D:, 1] = 1
    shared["blk2"] = blk2

    def head_mask(w):  # [2, 128] with per-head norm weight
        m = np.zeros((2, P), np.float32)
        m[0, :D] = w
        m[1, D:] = w
        return m.astype(BF).copy()

    shared["mq_ca"] = head_mask(np.asarray(inputs["ca_qn"], np.float32))
    shared["mk_ca"] = head_mask(np.asarray(inputs["ca_kn"], np.float32))
    shared["mq_sa"] = head_mask(np.asarray(inputs["sa_qn"], np.float32))
    shared["mk_sa"] = head_mask(np.asarray(inputs["sa_kn"], np.float32))

    r64 = np.zeros((D, D), np.float32)
    half = D // 2
    for j in range(half):
        r64[j, j + half] = -1.0  # rot[j] = -x[j+32]
        r64[j + half, j] = 1.0  # rot[j+32] = x[j]
    rt = r64.T  # lhsT (matmul computes lhsT.T @ rhs)
    rotm = np.zeros((P, P), np.float32)
    rotm[:D, :D] = rt
    rotm[D:, D:] = rt
    shared["rotm"] = rotm.astype(BF).copy()

    shared["ones_c"] = np.ones((P, 1), BF)
    shared["ones_r128"] = np.ones((1, P), BF)

    in_maps = []
    for c in range(NCORES):
        s, part = c // NR, c % NR
        rows = slice(part * TOK, (part + 1) * TOK)
        m = dict(shared)
        m["tgtT"] = _featmajor(tgt[s, rows])
        m["srcTb"] = _featmajor(src[s, rows]).astype(BF)
        cq, sq_ = _rope_tables(tgt_pos[s, rows])
        ck, sk = _rope_tables(src_pos[s, rows])
        m["cosq"], m["sinq"] = cq, sq_
        m["coskca"], m["sinkca"] = ck, sk
        in_maps.append(m)
    return in_maps


def _get_nc():
    if "nc" not in _cache:
        _cache["nc"] = _build_bass()
    return _cache["nc"]


def run(inputs, trace=False):
    """Run on 8 cores; returns (full_output, exec_time_ns_or_None)."""
    if trace:
        _install_ntff_hook()
    from concourse.bass_utils import run_bass_kernel_spmd

    in_maps = _prep_inputs(inputs)
    nc = _get_nc()
    res = run_bass_kernel_spmd(
        nc, in_maps, core_ids=list(range(NCORES)), trace=trace
    )
    out = np.empty((B, N, DIM), np.float32)
    for c in range(NCORES):
        s, part = c // NR, c % NR
        arr = np.asarray(res.results[c]["outT"])  # [128, 8, TOK]
        rows = slice(part * TOK, (part + 1) * TOK)
        out[s, rows] = np.transpose(arr, (2, 1, 0)).reshape(TOK, DIM)
    return out, res.exec_time_ns


def kernel(**inputs):
    out, _ = run(inputs, trace=False)
    return out


# revision 17
# speedup vs baseline: 1.2351x; 1.0581x over previous
"""Trainium2 Bass kernel for nn_CrossLayer (dense transformer layer).

Sharding: sequence-parallel over 8 cores (2 samples x 4 token-chunks of 512).
Each core computes its 512 token rows through CA -> SA -> FFN. K/V for all 16
heads are computed from each core's own rows (fp8) and AllGather'd across the
4 cores of its sample as two collectives (k first, then v) launched as early
as possible; q-projection and score/exp run-ahead hide the transfer.

On-chip layout: activations feature-major [dim(128p x 8c), tok] so every
matmul contracts over partitions. RMSNorm partition-sums via ones-matmuls on
PE; RoPE rotate-half via a constant +-1 block matrix on PE; softmax
denominators ride in row 64 of the av accumulators (ones column of v) and are
reciprocal'd on DVE + broadcast through a tiny DRAM bounce per head-pair,
overlapped with the next head's scores. exp uses bias=-2 so fp8/bf16 prob
tiles stay in range (cancels in the normalization).
"""

import math
import sys
import types

import numpy as np
import ml_dtypes

B, N, DIM, HID, H, D = 2, 2048, 1024, 4096, 16, 64
TOK = 512  # tokens per core
NCORES = 8
EPS = 1e-6
THETA = 10000.0
P = 128
KO = DIM // P  # 8 contraction chunks
HH = H // 2  # 8 head pairs
HC = HID // P  # 32 hidden chunks
TC = TOK // P  # 4 token chunks per core
NR = 4  # ranks per replica group
VW = D + 1  # v columns + ones column

BF = ml_dtypes.bfloat16

_cache = {}


def _lhsT_layout(W):
    """[K, M] -> [M//128, 128(K%128), K//128, 128(M%128)]: SBUF slices are
    matmul lhsT tiles [128, 128]."""
    K, M = W.shape
    return (
        W.reshape(K // P, P, M // P, P).transpose(2, 1, 0, 3).astype(BF).copy()
    )


def _rhs_layout(W):
    """[K, M] -> [128, K//128, M] rhs-style."""
    K, M = W.shape
    return W.reshape(K // P, P, M).transpose(1, 0, 2).astype(BF).copy()


def _featmajor(x):
    """[tok, dim] -> [128, dim//128, tok] float32."""
    return x.T.reshape(DIM // P, P, x.shape[0]).transpose(1, 0, 2).copy()


def _rope_tables(pos):
    """pos [TOK] int32 -> cos/sin [128, TOK] (2 heads stacked) bf16."""
    invf = 1.0 / (THETA ** (np.arange(0, D, 2, dtype=np.float64) / D))  # [32]
    ang = pos.astype(np.float64)[None, :] * invf[:, None]  # [32, TOK]
    c = np.cos(ang)
    s = np.sin(ang)
    c64 = np.concatenate([c, c], axis=0)  # [64, TOK]
    s64 = np.concatenate([s, s], axis=0)
    c128 = np.concatenate([c64, c64], axis=0).astype(BF)  # [128, TOK]
    s128 = np.concatenate([s64, s64], axis=0).astype(BF)
    return c128.copy(), s128.copy()


def _install_ntff_hook():
    try:
        from trn_agent_boot.trn_boot import _ntff_profile_via_ctypes
    except ImportError:
        return
    if "antenv.axon_hooks" in sys.modules:
        return
    try:
        hook = _ntff_profile_via_ctypes("/opt/axon/libaxon_pjrt.so")
    except OSError:
        return
    mod = types.ModuleType("antenv.axon_hooks")
    mod.get_axon_ntff_profile_hook = lambda: hook
    mod.set_axon_ntff_profile_hook = lambda h: None
    sys.modules["antenv.axon_hooks"] = mod
    import antenv

    antenv.axon_hooks = mod


def _split_multiwait(nc):
    """This walrus only supports one sync-wait on CTRL-encoded instructions
    (Drain/NoOp); hoist excess waits onto single-wait NoOps placed before."""
    from concourse import mybir

    n_split = 0
    for f in nc.m.functions:
        for bb in f.blocks:
            new = []
            changed = False
            for ins in bb.instructions:
                si = ins.sync_info
                if (
                    si is not None
                    and si.on_wait is not None
                    and len(si.on_wait) > 1
                ):
                    waits = list(si.on_wait)
                    keep, rest = waits[:1], waits[1:]
                    for k, w in enumerate(rest):
                        new.append(
                            mybir.InstNoOp(
                                name=f"{ins.name}-wsplit{k}",
                                engine=ins.engine,
                                sync_info=mybir.SyncInfo(
                                    on_wait=[w], on_update=[]
                                ),
                                bass_nofuse=True,
                            )
                        )
                    si.on_wait = keep
                    n_split += 1
                    changed = True
                new.append(ins)
            if changed:
                bb.instructions = new
    return n_split


def _build_bass():
    from contextlib import ExitStack

    import concourse.bass as bass
    import concourse.tile as tile
    from concourse import mybir

    f32 = mybir.dt.float32
    bf16 = mybir.dt.bfloat16
    f8 = mybir.dt.float8e4
    AF = mybir.ActivationFunctionType

    nc = bass.Bass(num_devices=NCORES)

    def inp(name, shape, dt=bf16):
        return nc.dram_tensor(name, shape, dt, kind="ExternalInput")

    tgtT = inp("tgtT", [P, KO, TOK], f32)
    srcTb = inp("srcTb", [P, KO, TOK])
    cosq = inp("cosq", [P, TOK])
    sinq = inp("sinq", [P, TOK])
    coskca = inp("coskca", [P, TOK])
    sinkca = inp("sinkca", [P, TOK])
    caWq = inp("caWq", [HH, P, KO, P])
    caWk = inp("caWk", [HH, P, KO, P])
    caWv = inp("caWv", [P, KO, DIM])
    caWo = inp("caWo", [KO, P, KO, P])
    saWq = inp("saWq", [HH, P, KO, P])
    saWk = inp("saWk", [HH, P, KO, P])
    saWv = inp("saWv", [P, KO, DIM])
    saWo = inp("saWo", [KO, P, KO, P])
    W1i = inp("W1", [HC, P, KO, P])
    W3i = inp("W3", [HC, P, KO, P])
    W2i = inp("W2", [KO, P, HC, P])
    blk2 = inp("blk2", [P, 2])  # per-head ssq lhsT (block ones)
    mq_ca = inp("mq_ca", [2, P])  # rsqrt bcast lhsT with qn folded
    mk_ca = inp("mk_ca", [2, P])
    mq_sa = inp("mq_sa", [2, P])
    mk_sa = inp("mk_sa", [2, P])
    rotm = inp("rotm", [P, P])  # rotate-half (2-head block diag) lhsT
    ones_c = inp("ones_c", [P, 1])  # y-norm ssq lhsT
    ones_r128 = inp("ones_r128", [1, P])  # y-norm bcast lhsT

    outT = nc.dram_tensor("outT", [P, KO, TOK], f32, kind="ExternalOutput")

    groups = [[0, 1, 2, 3], [4, 5, 6, 7]]
    KWORDS = P * HH * TOK  # k fp8 bytes per rank
    VWORDS = P * TC * H * VW  # v fp8 bytes per rank

    with tile.TileContext(nc) as tc:
        ctx = ExitStack()
        with ctx, nc.allow_low_precision("bf16/fp8 intermediates by design"):
            sing = ctx.enter_context(tc.tile_pool(name="sing", bufs=1))
            wpool = ctx.enter_context(tc.tile_pool(name="wpool", bufs=3))
            w2pool = ctx.enter_context(tc.tile_pool(name="w2pool", bufs=2))
            work = ctx.enter_context(tc.tile_pool(name="work", bufs=3))
            stat = ctx.enter_context(tc.tile_pool(name="stat", bufs=3))
            probp = ctx.enter_context(tc.tile_pool(name="probp", bufs=20))
            rpool = ctx.enter_context(tc.tile_pool(name="rpool", bufs=2))
            drpool = ctx.enter_context(tc.tile_pool(name="drpool", bufs=2))
            kvpool = ctx.enter_context(tc.tile_pool(name="kvpool", bufs=1))
            dram = ctx.enter_context(
                tc.tile_pool(name="dram", bufs=1, space="DRAM")
            )
            dbp = ctx.enter_context(
                tc.tile_pool(name="dbp", bufs=2, space="DRAM")
            )
            pp = ctx.enter_context(tc.tile_pool(name="pp", bufs=2, space="PSUM"))
            ps_s = ctx.enter_context(
                tc.tile_pool(name="ps_s", bufs=2, space="PSUM")
            )
            ps_x = ctx.enter_context(
                tc.tile_pool(name="ps_x", bufs=1, space="PSUM")
            )

            # ---- resident tiles
            srcT_sb = kvpool.tile([P, KO, TOK], bf16, tag="xT", name="srcT_sb")
            nc.sync.dma_start(srcT_sb[:], srcTb[:])
            cosq_sb = sing.tile([P, TOK], bf16)
            nc.sync.dma_start(cosq_sb[:], cosq[:])
            sinq_sb = sing.tile([P, TOK], bf16)
            nc.sync.dma_start(sinq_sb[:], sinq[:])
            coskca_sb = sing.tile([P, TOK], bf16)
            nc.sync.dma_start(coskca_sb[:], coskca[:])
            sinkca_sb = sing.tile([P, TOK], bf16)
            nc.sync.dma_start(sinkca_sb[:], sinkca[:])
            blk2_sb = sing.tile([P, 2], bf16)
            nc.sync.dma_start(blk2_sb[:], blk2[:])
            masks_sb = {}
            for name, t in (
                ("mq_ca", mq_ca),
                ("mk_ca", mk_ca),
                ("mq_sa", mq_sa),
                ("mk_sa", mk_sa),
            ):
                m = sing.tile([2, P], bf16, name=name)
                nc.sync.dma_start(m[:], t[:])
                masks_sb[name] = m
            rotm_sb = sing.tile([P, P], bf16)
            nc.sync.dma_start(rotm_sb[:], rotm[:])
            ones_c_sb = sing.tile([P, 1], bf16)
            nc.sync.dma_start(ones_c_sb[:], ones_c[:])
            ones_r128_sb = sing.tile([1, P], bf16)
            nc.sync.dma_start(ones_r128_sb[:], ones_r128[:])
            eps_sb = sing.tile([2, 1], mybir.dt.float32)
            nc.vector.memset(eps_sb[:], float(EPS))
            negb_sb = sing.tile([P, 1], mybir.dt.float32)
            nc.vector.memset(negb_sb[:], -2.0)
            resid = sing.tile([P, KO, TOK], f32)
            nc.sync.dma_start(resid[:], tgtT[:])

            def norm_rope_one(psum_q, mask_sb, cos_sb, sin_sb, dst):
                """psum_q [128(2 heads), TOK] f32 -> dst fp8: rms-normed,
                qn-scaled, roped."""
                raw = stat.tile([P, TOK], bf16, tag="raw", name="raw")
                nc.vector.tensor_copy(raw[:], psum_q[:])
                sq = work.tile([P, TOK], bf16, tag="ysq", name="sq")
                nc.vector.tensor_mul(sq[:], raw[:], raw[:])
                ssq = ps_s.tile([2, TOK], f32, tag="ps", name="ssq")
                nc.tensor.matmul(ssq[:], blk2_sb[:], sq[:], start=True, stop=True)
                # rsqrt(mean+eps) = exp(-0.5*ln(mean+eps)); Ln/Exp share one
                # ACT table set (natural_log_exp) with the attention exps
                lnt = stat.tile([2, TOK], f32, tag="lnt", name="lnt")
                nc.scalar.activation(
                    lnt[:], ssq[:], AF.Ln, bias=eps_sb[:], scale=1.0 / D
                )
                rs = stat.tile([2, TOK], bf16, tag="rs", name="rs")
                nc.scalar.activation(rs[:], lnt[:], AF.Exp, scale=-0.5)
                bc = ps_s.tile([P, TOK], f32, tag="ps", name="bc")
                nc.tensor.matmul(bc[:], mask_sb[:], rs[:], start=True, stop=True)
                v1 = stat.tile([P, TOK], bf16, tag="v1", name="v1")
                nc.vector.tensor_mul(v1[:], raw[:], bc[:])
                rot_ps = ps_s.tile([P, TOK], f32, tag="ps", name="rot_ps")
                nc.tensor.matmul(
                    rot_ps[:], rotm_sb[:], v1[:], start=True, stop=True
                )
                t1 = stat.tile([P, TOK], bf16, tag="t1", name="t1")
                nc.vector.tensor_mul(t1[:], v1[:], cos_sb[:])
                t2 = stat.tile([P, TOK], bf16, tag="t2", name="t2")
                nc.vector.tensor_mul(t2[:], rot_ps[:], sin_sb[:])
                nc.vector.tensor_add(dst, t1[:], t2[:])

            def rmsnorm_feat(src_f32, dst_bf16):
                """Feature-major RMSNorm: dst = src * rsqrt(mean(src^2))."""
                ssq = ps_s.tile([1, TOK], f32, tag="ps", name="yssq")
                for c in range(KO):
                    sq = work.tile([P, TOK], bf16, tag="ysq", name="ynsq")
                    nc.vector.tensor_mul(sq[:], src_f32[:, c], src_f32[:, c])
                    nc.tensor.matmul(
                        ssq[:],
                        ones_c_sb[:],
                        sq[:],
                        start=(c == 0),
                        stop=(c == KO - 1),
                    )
                lnt = stat.tile([1, TOK], f32, tag="lnt", name="ylnt")
                nc.scalar.activation(
                    lnt[:], ssq[:], AF.Ln, bias=eps_sb[:1], scale=1.0 / DIM
                )
                rs = stat.tile([1, TOK], bf16, tag="rs", name="yrs")
                nc.scalar.activation(rs[:], lnt[:], AF.Exp, scale=-0.5)
                bc = ps_s.tile([P, TOK], f32, tag="ps", name="ybc")
                nc.tensor.matmul(
                    bc[:], ones_r128_sb[:], rs[:], start=True, stop=True
                )
                for c in range(KO):
                    nc.vector.tensor_mul(dst_bf16[:, c], src_f32[:, c], bc[:])

            def proj_heads(Wt, src_sb, dst, mask, cos_sb, sin_sb, wname):
                """Project 16 heads (8 pairs) + rmsnorm + rope into dst fp8."""
                for g in range(2):
                    w = wpool.tile([P, 4, KO, P], bf16, tag="w1m", name=wname)
                    nc.sync.dma_start(
                        w[:],
                        Wt[g * 4 : (g + 1) * 4].rearrange(
                            "g p ko m -> p g ko m"
                        ),
                    )
                    for j in range(4):
                        hh = g * 4 + j
                        pq = pp.tile([P, TOK], f32, tag="pp", name="pq")
                        for c in range(KO):
                            nc.tensor.matmul(
                                pq[:],
                                w[:, j, c],
                                src_sb[:, c],
                                start=(c == 0),
                                stop=(c == KO - 1),
                            )
                        norm_rope_one(pq, mask, cos_sb, sin_sb, dst[:, hh])

            def attention_block(kvsrc_sb, Wq_t, Wk_t, Wv_t, Wo_t,
                                mq, mk, cosk, sink, make_y, blkname):
                """One attention block. kvsrc_sb bf16 [P,KO,TOK] is the kv-side
                input; make_y() returns the q-side input (emitted after the
                collectives launch). Adds Wo output into resid."""
                # --- k projection + norm/rope from my rows (fp8), gather early
                k_mine = kvpool.tile([P, HH, TOK], f8, tag="kq", name="k_mine")
                proj_heads(Wk_t, kvsrc_sb, k_mine, mk, cosk, sink, "wk")
                kin_k = dram.tile([KWORDS], f8, tag="kin_k")
                nc.sync.dma_start(
                    kin_k[:].rearrange("(p h t) -> p h t", p=P, h=HH, t=TOK),
                    k_mine[:],
                )
                kout_k = dram.tile([NR, KWORDS], f8, tag="kout_k")
                nc.gpsimd.collective_compute(
                    "AllGather",
                    mybir.AluOpType.bypass,
                    replica_groups=groups,
                    ins=[kin_k.opt()],
                    outs=[kout_k.opt()],
                )

                # --- v projection (token-major, with ones column), fp8
                v_mine = kvpool.tile(
                    [P, TC, H, VW], f8, tag="vm", name="v_mine"
                )
                nc.vector.memset(v_mine[:, :, :, D : D + 1], 1.0)
                for nh in range(2):
                    wv = wpool.tile([P, KO, TOK], bf16, tag="w1m", name="wv")
                    nc.sync.dma_start(
                        wv[:], Wv_t[:, :, nh * TOK : (nh + 1) * TOK]
                    )
                    for t4 in range(TC):
                        pv = pp.tile([P, TOK], f32, tag="pp", name="pv")
                        for c in range(KO):
                            nc.tensor.matmul(
                                pv[:],
                                kvsrc_sb[:, c, t4 * P : (t4 + 1) * P],
                                wv[:, c],
                                start=(c == 0),
                                stop=(c == KO - 1),
                            )
                        nc.vector.tensor_copy(
                            v_mine[:, t4, nh * 8 : (nh + 1) * 8, 0:D],
                            pv[:].rearrange("p (h d) -> p h d", d=D),
                        )
                kin_v = dram.tile([VWORDS], f8, tag="kin_v")
                nc.sync.dma_start(
                    kin_v[:].rearrange(
                        "(p a b c) -> p a b c", p=P, a=TC, b=H, c=VW
                    ),
                    v_mine[:],
                )
                kout_v = dram.tile([NR, VWORDS], f8, tag="kout_v")
                nc.gpsimd.collective_compute(
                    "AllGather",
                    mybir.AluOpType.bypass,
                    replica_groups=groups,
                    ins=[kin_v.opt()],
                    outs=[kout_v.opt()],
                )

                # --- q projection + norm + rope (overlaps the collectives)
                y_sb = make_y()
                q_sb = kvpool.tile([P, HH, TOK], f8, tag="kq", name="q_sb")
                proj_heads(Wq_t, y_sb, q_sb, mq, cosq_sb, sinq_sb, "wq")

                # --- unpack gathered k/v (emitted after q DMAs so the sync
                # queue doesn't block q-side work on the collective)
                k_all = kvpool.tile(
                    [P, HH, NR, TOK], f8, tag="k_all", name="k_all"
                )
                v_all = kvpool.tile(
                    [P, NR, TC, H, VW], f8, tag="v_all", name="v_all"
                )
                for r in range(NR):
                    nc.sync.dma_start(
                        k_all[:, :, r],
                        kout_k[r].rearrange(
                            "(p h t) -> p h t", p=P, h=HH, t=TOK
                        ),
                    )
                for r in range(NR):
                    nc.sync.dma_start(
                        v_all[:, r],
                        kout_v[r].rearrange(
                            "(p a b c) -> p a b c", p=P, a=TC, b=H, c=VW
                        ),
                    )

                # --- attention: per head-pair, all 16 score/exp chunks then
                # the 32 av matmuls (prob run-ahead absorbs collective
                # latency); denominators ride in row 64 of px (ones col of v)
                xT = kvpool.tile([P, HH, TOK], bf16, tag="xT", name="xT")
                # software pipeline: head-pair hh's scores+exp interleave with
                # head-pair hh-1's av matmuls so neither PE nor ACT drains
                prev = None  # (probs, px, hh-1)
                for hh in range(HH + 1):
                    probs = []
                    px = None
                    if hh < HH:
                        px = [
                            ps_x.tile(
                                [VW, TOK], f32, tag=f"px{i}", name=f"px{i}"
                            )
                            for i in range(2)
                        ]
                    for kc in range(H):  # 16 k-chunks of 128 tokens
                        r, tcl = kc // TC, kc % TC
                        if hh < HH:
                            ps = ps_s.tile(
                                [P, 2 * TOK], f32, tag="ps", name="ps"
                            )
                            for i in range(2):
                                off = i * D
                                nc.tensor.matmul(
                                    ps[:, i * TOK : (i + 1) * TOK],
                                    k_all[
                                        off : off + D,
                                        hh,
                                        r,
                                        tcl * P : (tcl + 1) * P,
                                    ],
                                    q_sb[off : off + D, hh],
                                    start=True,
                                    stop=True,
                                )
                            prob = probp.tile(
                                [P, 2 * TOK], bf16, tag="prob", name="prob"
                            )
                            # exp(s/sqrt(d) - 2): the -2 cancels against the
                            # denominator and keeps prob magnitudes small
                            nc.scalar.activation(
                                prob[:], ps[:], AF.Exp,
                                scale=1.0 / math.sqrt(D), bias=negb_sb[:],
                            )
                            probs.append((prob, r, tcl))
                        if prev is not None:
                            pprobs, ppx, phh = prev
                            pprob, r0, tcl0 = pprobs[kc]
                            for i in range(2):
                                h = phh * 2 + i
                                nc.tensor.matmul(
                                    ppx[i][:],
                                    v_all[:, r0, tcl0, h],
                                    pprob[:, i * TOK : (i + 1) * TOK],
                                    start=(kc == 0),
                                    stop=(kc == H - 1),
                                )
                    if prev is not None:
                        # --- softmax denominators for head pair phh:
                        # reciprocal on DVE, broadcast via a tiny DRAM bounce;
                        # overlaps head-pair hh's scores
                        pprobs, ppx, phh = prev
                        drec = drpool.tile(
                            [VW, 2, TOK], bf16, tag="drec", name="drec"
                        )
                        for i in range(2):
                            nc.vector.reciprocal(
                                drec[D : D + 1, i], ppx[i][D : D + 1]
                            )
                        db = dbp.tile([2 * TOK], bf16, tag="db")
                        nc.sync.dma_start(
                            db[:].rearrange("(o a t) -> o a t", o=1, a=2),
                            drec[D : D + 1],
                        )
                        rb = rpool.tile([P, TOK], bf16, tag="rb", name="rb")
                        for i in range(2):
                            src = bass.AP(
                                tensor=db.tensor,
                                offset=db.offset + i * TOK,
                                ap=[[0, D], [1, TOK]],
                            )
                            nc.sync.dma_start(rb[i * D : (i + 1) * D], src)
                            nc.vector.tensor_copy(
                                xT[i * D : (i + 1) * D, phh], ppx[i][0:D]
                            )
                        nc.vector.tensor_mul(xT[:, phh], xT[:, phh], rb[:])
                    prev = (probs, px, hh) if hh < HH else None

                # --- Wo projection, accumulate into resid
                for g in range(2):
                    wo = wpool.tile([P, 4, KO, P], bf16, tag="w1m", name="wo")
                    nc.sync.dma_start(
                        wo[:],
                        Wo_t[g * 4 : (g + 1) * 4].rearrange(
                            "g p ko m -> p g ko m"
                        ),
                    )
                    for j in range(4):
                        oc = g * 4 + j
                        po = pp.tile([P, TOK], f32, tag="pp", name="po")
                        for c in range(KO):
                            nc.tensor.matmul(
                                po[:],
                                wo[:, j, c],
                                xT[:, c],
                                start=(c == 0),
                                stop=(c == KO - 1),
                            )
                        nc.vector.tensor_add(resid[:, oc], resid[:, oc], po[:])

            # ================= cross-attention =================
            yT = sing.tile([P, KO, TOK], bf16, name="yT")

            def make_y_ca():
                rmsnorm_feat(resid, yT)
                return yT

            attention_block(
                srcT_sb, caWq, caWk, caWv, caWo,
                masks_sb["mq_ca"], masks_sb["mk_ca"], coskca_sb, sinkca_sb,
                make_y_ca, "ca",
            )

            # ================= self-attention =================
            rmsnorm_feat(resid, yT)
            attention_block(
                yT, saWq, saWk, saWv, saWo,
                masks_sb["mq_sa"], masks_sb["mk_sa"], cosq_sb, sinq_sb,
                lambda: yT, "sa",
            )

            # ================= FFN =================
            rmsnorm_feat(resid, yT)
            hT = kvpool.tile([P, HC, TOK], bf16, tag="k_all", name="hT")
            for g in range(8):  # stream W1/W3 in eighths
                w1 = wpool.tile([P, 4, KO, P], bf16, tag="w1m", name="w1")
                nc.sync.dma_start(
                    w1[:],
                    W1i[g * 4 : (g + 1) * 4].rearrange("g p ko m -> p g ko m"),
                )
                w3 = wpool.tile([P, 4, KO, P], bf16, tag="w1m", name="w3")
                nc.sync.dma_start(
                    w3[:],
                    W3i[g * 4 : (g + 1) * 4].rearrange("g p ko m -> p g ko m"),
                )
                for j in range(4):
                    hc = g * 4 + j
                    p1 = pp.tile([P, TOK], f32, tag="pp", name="p1")
                    for c in range(KO):
                        nc.tensor.matmul(
                            p1[:], w1[:, j, c], yT[:, c],
                            start=(c == 0), stop=(c == KO - 1),
                        )
                    p3 = ps_s.tile([P, TOK], f32, tag="ps", name="p3")
                    for c in range(KO):
                        nc.tensor.matmul(
                            p3[:], w3[:, j, c], yT[:, c],
                            start=(c == 0), stop=(c == KO - 1),
                        )
                    s1 = stat.tile([P, TOK], f32, tag="raw", name="s1")
                    nc.scalar.activation(s1[:], p1[:], AF.Silu)
                    nc.vector.tensor_mul(hT[:, hc], s1[:], p3[:])
            for oc in range(KO):
                w2 = w2pool.tile([P, HC, P], bf16, tag="w2", name="w2")
                for c4 in range(4):
                    nc.sync.dma_start(
                        w2[:, c4 * 8 : (c4 + 1) * 8],
                        W2i[oc, :, c4 * 8 : (c4 + 1) * 8],
                    )
                po = pp.tile([P, TOK], f32, tag="pp", name="po2")
                for hc in range(HC):
                    nc.tensor.matmul(
                        po[:], w2[:, hc], hT[:, hc],
                        start=(hc == 0), stop=(hc == HC - 1),
                    )
                nc.vector.tensor_add(resid[:, oc], resid[:, oc], po[:])

            nc.sync.dma_start(outT[:], resid[:])

    _split_multiwait(nc)
    return nc


def _prep_inputs(inputs):
    """Full problem inputs -> list of 8 per-core in_maps."""
    tgt = np.asarray(inputs["tgt"], np.float32)
    src = np.asarray(inputs["src"], np.float32)
    tgt_pos = np.asarray(inputs["tgt_pos"], np.int32)
    src_pos = np.asarray(inputs["src_pos"], np.int32)

    pre_ca_w = np.asarray(inputs["pre_ca_w"], np.float32)
    pre_sa_w = np.asarray(inputs["pre_sa_w"], np.float32)
    pre_ffn_w = np.asarray(inputs["pre_ffn_w"], np.float32)

    def fold(Wname, w):
        return np.asarray(inputs[Wname], np.float32) * w[:, None]

    ca_Wq = fold("ca_Wq", pre_ca_w)
    ca_Wkv = np.asarray(inputs["ca_Wkv"], np.float32)
    ca_Wk, ca_Wv = ca_Wkv[:, :DIM], ca_Wkv[:, DIM:]
    ca_Wo = np.asarray(inputs["ca_Wo"], np.float32)
    sa_Wq = fold("sa_Wq", pre_sa_w)
    sa_Wkv = fold("sa_Wkv", pre_sa_w)
    sa_Wk, sa_Wv = sa_Wkv[:, :DIM], sa_Wkv[:, DIM:]
    sa_Wo = np.asarray(inputs["sa_Wo"], np.float32)
    W1 = fold("W1", pre_ffn_w)
    W3 = fold("W3", pre_ffn_w)
    W2 = np.asarray(inputs["W2"], np.float32)

    shared = {
        "caWq": _lhsT_layout(ca_Wq),
        "caWk": _lhsT_layout(ca_Wk),
        "caWv": _rhs_layout(ca_Wv),
        "caWo": _lhsT_layout(ca_Wo),
        "saWq": _lhsT_layout(sa_Wq),
        "saWk": _lhsT_layout(sa_Wk),
        "saWv": _rhs_layout(sa_Wv),
        "saWo": _lhsT_layout(sa_Wo),
        "W1": _lhsT_layout(W1),
        "W3": _lhsT_layout(W3),
        "W2": _lhsT_layout(W2),
    }

    blk2 = np.zeros((P, 2), BF)
    blk2[:D, 0] = 1
    blk2[D:, 1] = 1
    shared["blk2"] = blk2

    def head_mask(w):  # [2, 128] with per-head norm weight
        m = np.zeros((2, P), np.float32)
        m[0, :D] = w
        m[1, D:] = w
        return m.astype(BF).copy()

    shared["mq_ca"] = head_mask(np.asarray(inputs["ca_qn"], np.float32))
    shared["mk_ca"] = head_mask(np.asarray(inputs["ca_kn"], np.float32))
    shared["mq_sa"] = head_mask(np.asarray(inputs["sa_qn"], np.float32))
    shared["mk_sa"] = head_mask(np.asarray(inputs["sa_kn"], np.float32))

    r64 = np.zeros((D, D), np.float32)
    half = D // 2
    for j in range(half):
        r64[j, j + half] = -1.0  # rot[j] = -x[j+32]
        r64[j + half, j] = 1.0  # rot[j+32] = x[j]
    rt = r64.T  # lhsT (matmul computes lhsT.T @ rhs)
    rotm = np.zeros((P, P), np.float32)
    rotm[:D, :D] = rt
    rotm[D:, D:] = rt
    shared["rotm"] = rotm.astype(BF).copy()

    shared["ones_c"] = np.ones((P, 1), BF)
    shared["ones_r128"] = np.ones((1, P), BF)

    in_maps = []
    for c in range(NCORES):
        s, part = c // NR, c % NR
        rows = slice(part * TOK, (part + 1) * TOK)
        m = dict(shared)
        m["tgtT"] = _featmajor(tgt[s, rows])
        m["srcTb"] = _featmajor(src[s, rows]).astype(BF)
        cq, sq_ = _rope_tables(tgt_pos[s, rows])
        ck, sk = _rope_tables(src_pos[s, rows])
        m["cosq"], m["sinq"] = cq, sq_
        m["coskca"], m["sinkca"] = ck, sk
        in_maps.append(m)
    return in_maps


def _get_nc():
    if "nc" not in _cache:
        _cache["nc"] = _build_bass()
    return _cache["nc"]


def run(inputs, trace=False):
    """Run on 8 cores; returns (full_output, exec_time_ns_or_None)."""
    if trace:
        _install_ntff_hook()
    from concourse.bass_utils import run_bass_kernel_spmd

    in_maps = _prep_inputs(inputs)
    nc = _get_nc()
    res = run_bass_kernel_spmd(
        nc, in_maps, core_ids=list(range(NCORES)), trace=trace
    )
    out = np.empty((B, N, DIM), np.float32)
    for c in range(NCORES):
        s, part = c // NR, c % NR
        arr = np.asarray(res.results[c]["outT"])  # [128, 8, TOK]
        rows = slice(part * TOK, (part + 1) * TOK)
        out[s, rows] = np.transpose(arr, (2, 1, 0)).reshape(TOK, DIM)
    return out, res.exec_time_ns


def kernel(**inputs):
    out, _ = run(inputs, trace=False)
    return out


# revision 32
# speedup vs baseline: 1.6739x; 1.3553x over previous
"""Trainium2 Bass kernel for nn_CrossLayer (dense transformer layer).

Sharding: sequence-parallel over 8 cores (2 samples x 4 token-chunks of 512).
Each core computes its 512 token rows through CA -> SA -> FFN. K/V for all 16
heads are computed from each core's own rows (fp8) and AllGather'd across the
4 cores of its sample as two collectives (k first, then v) launched as early
as possible; q-projection and score/exp run-ahead hide the transfer.

On-chip layout: activations feature-major [dim(128p x 8c), tok] so every
matmul contracts over partitions. RMSNorm partition-sums via ones-matmuls on
PE; RoPE rotate-half via a constant +-1 block matrix on PE; softmax
denominators ride in row 64 of the av accumulators (ones column of v) and are
reciprocal'd on DVE + broadcast through a tiny DRAM bounce per head-pair,
overlapped with the next head's scores. exp uses bias=-2 so fp8/bf16 prob
tiles stay in range (cancels in the normalization).
"""

import math
import sys
import types

import numpy as np
import ml_dtypes

B, N, DIM, HID, H, D = 2, 2048, 1024, 4096, 16, 64
TOK = 512  # tokens per core
NCORES = 8
EPS = 1e-6
THETA = 10000.0
P = 128
KO = DIM // P  # 8 contraction chunks
HH = H // 2  # 8 head pairs
HC = HID // P  # 32 hidden chunks
TC = TOK // P  # 4 token chunks per core
NR = 4  # ranks per replica group
VW = D + 1  # v columns + ones column

BF = ml_dtypes.bfloat16
F8 = ml_dtypes.float8_e4m3  # matches TRN FP8_EXP4 (max 240, has inf)

SW = 512.0  # weight quant scale (Wq/Wk/Wo/W1/W3/W2)
SV = 32.0  # Wv quant scale
SH = 16.0  # hT quant scale

_cache = {}


def _lhsT_layout(W):
    """[K, M] -> [M//128, 128(K%128), K//128, 128(M%128)]: SBUF slices are
    matmul lhsT tiles [128, 128]."""
    K, M = W.shape
    return (
        W.reshape(K // P, P, M // P, P).transpose(2, 1, 0, 3).astype(BF).copy()
    )


def _rhs_layout(W):
    """[K, M] -> [128, K//128, M] rhs-style."""
    K, M = W.shape
    return W.reshape(K // P, P, M).transpose(1, 0, 2).astype(BF).copy()


def _lhsT_f8(W, s):
    """fp8 lhsT layout, scaled by s and clipped to the TRN e4m3 range."""
    K, M = W.shape
    Ws = np.clip(W * s, -240.0, 240.0)
    return (
        Ws.reshape(K // P, P, M // P, P).transpose(2, 1, 0, 3).astype(F8).copy()
    )


def _rhs_f8(W, s):
    K, M = W.shape
    Ws = np.clip(W * s, -240.0, 240.0)
    return Ws.reshape(K // P, P, M).transpose(1, 0, 2).astype(F8).copy()


def _featmajor(x):
    """[tok, dim] -> [128, dim//128, tok] float32."""
    return x.T.reshape(DIM // P, P, x.shape[0]).transpose(1, 0, 2).copy()


def _rope_tables(pos):
    """pos [TOK] int32 -> cos/sin [128, TOK] (2 heads stacked) bf16."""
    invf = 1.0 / (THETA ** (np.arange(0, D, 2, dtype=np.float64) / D))  # [32]
    ang = pos.astype(np.float64)[None, :] * invf[:, None]  # [32, TOK]
    c = np.cos(ang)
    s = np.sin(ang)
    c64 = np.concatenate([c, c], axis=0)  # [64, TOK]
    s64 = np.concatenate([s, s], axis=0)
    c128 = np.concatenate([c64, c64], axis=0).astype(BF)  # [128, TOK]
    s128 = np.concatenate([s64, s64], axis=0).astype(BF)
    return c128.copy(), s128.copy()


def _install_ntff_hook():
    try:
        from trn_agent_boot.trn_boot import _ntff_profile_via_ctypes
    except ImportError:
        return
    if "antenv.axon_hooks" in sys.modules:
        return
    try:
        hook = _ntff_profile_via_ctypes("/opt/axon/libaxon_pjrt.so")
    except OSError:
        return
    mod = types.ModuleType("antenv.axon_hooks")
    mod.get_axon_ntff_profile_hook = lambda: hook
    mod.set_axon_ntff_profile_hook = lambda h: None
    sys.modules["antenv.axon_hooks"] = mod
    import antenv

    antenv.axon_hooks = mod


def _split_multiwait(nc):
    """This walrus only supports one sync-wait on CTRL-encoded instructions
    (Drain/NoOp); hoist excess waits onto single-wait NoOps placed before."""
    from concourse import mybir

    n_split = 0
    for f in nc.m.functions:
        for bb in f.blocks:
            new = []
            changed = False
            for ins in bb.instructions:
                si = ins.sync_info
                if (
                    si is not None
                    and si.on_wait is not None
                    and len(si.on_wait) > 1
                ):
                    waits = list(si.on_wait)
                    keep, rest = waits[:1], waits[1:]
                    for k, w in enumerate(rest):
                        new.append(
                            mybir.InstNoOp(
                                name=f"{ins.name}-wsplit{k}",
                                engine=ins.engine,
                                sync_info=mybir.SyncInfo(
                                    on_wait=[w], on_update=[]
                                ),
                                bass_nofuse=True,
                            )
                        )
                    si.on_wait = keep
                    n_split += 1
                    changed = True
                new.append(ins)
            if changed:
                bb.instructions = new
    return n_split


def _build_bass():
    from contextlib import ExitStack

    import concourse.bass as bass
    import concourse.tile as tile
    from concourse import mybir

    f32 = mybir.dt.float32
    bf16 = mybir.dt.bfloat16
    f8 = mybir.dt.float8e4
    AF = mybir.ActivationFunctionType

    nc = bass.Bass(num_devices=NCORES)

    def inp(name, shape, dt=bf16):
        return nc.dram_tensor(name, shape, dt, kind="ExternalInput")

    tgtT = inp("tgtT", [P, KO, TOK], f32)
    srcTb = inp("srcTb", [P, KO, TOK], f8)
    cosq = inp("cosq", [P, TOK])
    sinq = inp("sinq", [P, TOK])
    coskca = inp("coskca", [P, TOK])
    sinkca = inp("sinkca", [P, TOK])
    caWq = inp("caWq", [HH, P, KO, P], f8)
    caWk = inp("caWk", [HH, P, KO, P], f8)
    caWv = inp("caWv", [P, KO, DIM], f8)
    caWo = inp("caWo", [KO, P, KO, P], f8)
    saWq = inp("saWq", [HH, P, KO, P], f8)
    saWk = inp("saWk", [HH, P, KO, P], f8)
    saWv = inp("saWv", [P, KO, DIM], f8)
    saWo = inp("saWo", [KO, P, KO, P], f8)
    W1i = inp("W1", [HC, P, KO, P], f8)
    W3i = inp("W3", [HC, P, KO, P], f8)
    W2i = inp("W2", [KO, P, HC, P], f8)
    blk2 = inp("blk2", [P, 2])  # per-head ssq lhsT (block ones)
    mq_ca = inp("mq_ca", [2, P])  # rsqrt bcast lhsT with qn folded
    mk_ca = inp("mk_ca", [2, P])
    mq_sa = inp("mq_sa", [2, P])
    mk_sa = inp("mk_sa", [2, P])
    rotm = inp("rotm", [P, P])  # rotate-half (2-head block diag) lhsT
    ones_c = inp("ones_c", [P, 1])  # y-norm ssq lhsT
    ones_r128 = inp("ones_r128", [1, P])  # y-norm bcast lhsT

    outT = nc.dram_tensor("outT", [P, KO, TOK], f32, kind="ExternalOutput")

    groups = [[0, 1, 2, 3], [4, 5, 6, 7]]
    KWORDS = P * HH * TOK  # k fp8 bytes per rank
    VWORDS = P * TC * H * VW  # v fp8 bytes per rank

    with tile.TileContext(nc) as tc:
        ctx = ExitStack()
        with ctx, nc.allow_low_precision("bf16/fp8 intermediates by design"):
            sing = ctx.enter_context(tc.tile_pool(name="sing", bufs=1))
            wpool = ctx.enter_context(tc.tile_pool(name="wpool", bufs=3))
            w2pool = ctx.enter_context(tc.tile_pool(name="w2pool", bufs=2))
            work = ctx.enter_context(tc.tile_pool(name="work", bufs=3))
            stat = ctx.enter_context(tc.tile_pool(name="stat", bufs=4))
            probp = ctx.enter_context(tc.tile_pool(name="probp", bufs=12))
            rpool = ctx.enter_context(tc.tile_pool(name="rpool", bufs=2))
            drpool = ctx.enter_context(tc.tile_pool(name="drpool", bufs=2))
            kvpool = ctx.enter_context(tc.tile_pool(name="kvpool", bufs=1))
            dram = ctx.enter_context(
                tc.tile_pool(name="dram", bufs=1, space="DRAM")
            )
            dbp = ctx.enter_context(
                tc.tile_pool(name="dbp", bufs=2, space="DRAM")
            )
            pp = ctx.enter_context(tc.tile_pool(name="pp", bufs=2, space="PSUM"))
            ps_s = ctx.enter_context(
                tc.tile_pool(name="ps_s", bufs=2, space="PSUM")
            )
            ps_x = ctx.enter_context(
                tc.tile_pool(name="ps_x", bufs=1, space="PSUM")
            )

            # ---- resident tiles
            srcT_sb = kvpool.tile([P, KO, TOK], f8, tag="xT", name="srcT_sb")
            nc.sync.dma_start(srcT_sb[:], srcTb[:])
            cosq_sb = sing.tile([P, TOK], bf16)
            nc.sync.dma_start(cosq_sb[:], cosq[:])
            sinq_sb = sing.tile([P, TOK], bf16)
            nc.sync.dma_start(sinq_sb[:], sinq[:])
            coskca_sb = sing.tile([P, TOK], bf16)
            nc.sync.dma_start(coskca_sb[:], coskca[:])
            sinkca_sb = sing.tile([P, TOK], bf16)
            nc.sync.dma_start(sinkca_sb[:], sinkca[:])
            blk2_sb = sing.tile([P, 2], bf16)
            nc.sync.dma_start(blk2_sb[:], blk2[:])
            masks_sb = {}
            for name, t in (
                ("mq_ca", mq_ca),
                ("mk_ca", mk_ca),
                ("mq_sa", mq_sa),
                ("mk_sa", mk_sa),
            ):
                m = sing.tile([2, P], bf16, name=name)
                nc.sync.dma_start(m[:], t[:])
                masks_sb[name] = m
            rotm_sb = sing.tile([P, P], bf16)
            nc.sync.dma_start(rotm_sb[:], rotm[:])
            ones_c_sb = sing.tile([P, 1], bf16)
            nc.sync.dma_start(ones_c_sb[:], ones_c[:])
            ones_r128_sb = sing.tile([1, P], bf16)
            nc.sync.dma_start(ones_r128_sb[:], ones_r128[:])
            eps_sb = sing.tile([2, 1], mybir.dt.float32)
            nc.vector.memset(eps_sb[:], float(EPS))
            negb_sb = sing.tile([P, 1], mybir.dt.float32)
            nc.vector.memset(negb_sb[:], -2.0)
            resid = sing.tile([P, KO, TOK], f32)
            nc.sync.dma_start(resid[:], tgtT[:])

            DR = mybir.MatmulPerfMode.DoubleRow

            def rmsnorm_feat(src_f32, dst_bf16):
                """Feature-major RMSNorm: dst = src * rsqrt(mean(src^2))."""
                ssq = ps_s.tile([1, TOK], f32, tag="ps", name="yssq")
                for c in range(KO):
                    sq = work.tile([P, TOK], bf16, tag="ysq", name="ynsq")
                    nc.vector.tensor_mul(sq[:], src_f32[:, c], src_f32[:, c])
                    nc.tensor.matmul(
                        ssq[:],
                        ones_c_sb[:],
                        sq[:],
                        start=(c == 0),
                        stop=(c == KO - 1),
                    )
                lnt = stat.tile([1, TOK], f32, tag="lnt", name="ylnt")
                nc.scalar.activation(
                    lnt[:], ssq[:], AF.Ln, bias=eps_sb[:1], scale=1.0 / DIM
                )
                rs = stat.tile([1, TOK], bf16, tag="rs", name="yrs")
                nc.scalar.activation(rs[:], lnt[:], AF.Exp, scale=-0.5)
                bc = ps_s.tile([P, TOK], f32, tag="ps", name="ybc")
                nc.tensor.matmul(
                    bc[:], ones_r128_sb[:], rs[:], start=True, stop=True
                )
                for c in range(KO):
                    nc.vector.tensor_mul(dst_bf16[:, c], src_f32[:, c], bc[:])

            def proj_heads(Wt, src_sb, dst, mask, cos_sb, sin_sb, wname):
                """Project 16 heads (8 pairs) + rmsnorm + rope into dst fp8.

                Stage-pipelined: each head-pair's norm/rope chain PE ops lag
                its projection burst by 1-3 slots so the in-order PE queue
                never waits on the ACT/DVE chain latency. Chain psums
                time-share the attention-phase slots (tags ps/px0/px1)."""
                st = {}

                def s0(j):  # free the proj psum, square
                    raw = stat.tile([P, TOK], bf16, tag="raw", name="raw")
                    nc.vector.tensor_copy(raw[:], st[j]["pq"][:])
                    sq = work.tile([P, TOK], bf16, tag="ysq", name="sq")
                    nc.vector.tensor_mul(sq[:], raw[:], raw[:])
                    st[j].update(raw=raw, sq=sq)

                def s1(j):  # per-head sum of squares -> rsqrt
                    ssq = ps_s.tile([2, TOK], f32, tag="ps", name="ssq")
                    nc.tensor.matmul(
                        ssq[:], blk2_sb[:], st[j]["sq"][:],
                        start=True, stop=True,
                    )
                    lnt = stat.tile([2, TOK], f32, tag="lnt", name="lnt")
                    nc.scalar.activation(
                        lnt[:], ssq[:], AF.Ln, bias=eps_sb[:], scale=1.0 / D
                    )
                    rs = stat.tile([2, TOK], bf16, tag="rs", name="rs")
                    nc.scalar.activation(rs[:], lnt[:], AF.Exp, scale=-0.5)
                    st[j]["rs"] = rs

                def s2(j):  # broadcast rsqrt, normalize, cos term
                    bc = ps_x.tile([P, TOK], f32, tag="px0", name="bc")
                    nc.tensor.matmul(
                        bc[:], mask[:], st[j]["rs"][:], start=True, stop=True
                    )
                    v1 = stat.tile([P, TOK], bf16, tag="v1", name="v1")
                    nc.vector.tensor_mul(v1[:], st[j]["raw"][:], bc[:])
                    t1 = stat.tile([P, TOK], bf16, tag="t1", name="t1")
                    nc.vector.tensor_mul(t1[:], v1[:], cos_sb[:])
                    st[j].update(v1=v1, t1=t1)

                def s3(j):  # rotate-half, sin term, combine
                    rot_ps = ps_x.tile([P, TOK], f32, tag="px1", name="rot_ps")
                    nc.tensor.matmul(
                        rot_ps[:], rotm_sb[:], st[j]["v1"][:],
                        start=True, stop=True,
                    )
                    t2 = stat.tile([P, TOK], bf16, tag="t2", name="t2")
                    nc.vector.tensor_mul(t2[:], rot_ps[:], sin_sb[:])
                    nc.vector.tensor_add(dst[:, j], st[j]["t1"][:], t2[:])
                    del st[j]

                w_tiles = {}
                for j in range(HH + 3):
                    if j < HH:
                        g, jj = j // 4, j % 4
                        if jj == 0:
                            w = wpool.tile(
                                [P, 4, KO, P], f8, tag="w1m", name=wname
                            )
                            nc.sync.dma_start(
                                w[:],
                                Wt[g * 4 : (g + 1) * 4].rearrange(
                                    "g p ko m -> p g ko m"
                                ),
                            )
                            w_tiles[g] = w
                        w = w_tiles[g]
                        pq = pp.tile([P, TOK], f32, tag="pp", name="pq")
                        for c2 in range(KO // 2):
                            nc.tensor.matmul(
                                pq[:],
                                w[:, jj, 2 * c2 : 2 * c2 + 2, :],
                                src_sb[:, 2 * c2 : 2 * c2 + 2, :],
                                perf_mode=DR,
                                start=(c2 == 0),
                                stop=(c2 == KO // 2 - 1),
                            )
                        st[j] = {"pq": pq}
                        s0(j)
                    if 0 <= j - 1 < HH:
                        s1(j - 1)
                    if 0 <= j - 2 < HH:
                        s2(j - 2)
                    if 0 <= j - 3 < HH:
                        s3(j - 3)

            def attention_block(kvsrc_sb, Wq_t, Wk_t, Wv_t, Wo_t,
                                mq, mk, cosk, sink, make_y, blkname):
                """One attention block. kvsrc_sb bf16 [P,KO,TOK] is the kv-side
                input; make_y() returns the q-side input (emitted after the
                collectives launch). Adds Wo output into resid."""
                # --- k projection + norm/rope from my rows (fp8), gather early
                k_mine = kvpool.tile([P, HH, TOK], f8, tag="kq", name="k_mine")
                proj_heads(Wk_t, kvsrc_sb, k_mine, mk, cosk, sink, "wk")
                kin_k = dram.tile([KWORDS], f8, tag="kin_k")
                nc.sync.dma_start(
                    kin_k[:].rearrange("(p h t) -> p h t", p=P, h=HH, t=TOK),
                    k_mine[:],
                )
                kout_k = dram.tile([NR, KWORDS], f8, tag="kout_k")
                nc.gpsimd.collective_compute(
                    "AllGather",
                    mybir.AluOpType.bypass,
                    replica_groups=groups,
                    ins=[kin_k.opt()],
                    outs=[kout_k.opt()],
                )

                # --- v projection (token-major, with ones column), fp8
                v_mine = kvpool.tile(
                    [P, TC, H, VW], f8, tag="vm", name="v_mine"
                )
                nc.vector.memset(v_mine[:, :, :, D : D + 1], 1.0)
                for nh in range(2):
                    wv = wpool.tile([P, KO, TOK], f8, tag="w1m", name="wv")
                    nc.sync.dma_start(
                        wv[:], Wv_t[:, :, nh * TOK : (nh + 1) * TOK]
                    )
                    for t4 in range(TC):
                        pv = pp.tile([P, TOK], f32, tag="pp", name="pv")
                        for c2 in range(KO // 2):
                            nc.tensor.matmul(
                                pv[:],
                                kvsrc_sb[
                                    :, 2 * c2 : 2 * c2 + 2,
                                    t4 * P : (t4 + 1) * P,
                                ],
                                wv[:, 2 * c2 : 2 * c2 + 2, :],
                                perf_mode=DR,
                                start=(c2 == 0),
                                stop=(c2 == KO // 2 - 1),
                            )
                        nc.vector.tensor_copy(
                            v_mine[:, t4, nh * 8 : (nh + 1) * 8, 0:D],
                            pv[:].rearrange("p (h d) -> p h d", d=D),
                        )
                kin_v = dram.tile([VWORDS], f8, tag="kin_v")
                nc.sync.dma_start(
                    kin_v[:].rearrange(
                        "(p a b c) -> p a b c", p=P, a=TC, b=H, c=VW
                    ),
                    v_mine[:],
                )
                kout_v = dram.tile([NR, VWORDS], f8, tag="kout_v")
                nc.gpsimd.collective_compute(
                    "AllGather",
                    mybir.AluOpType.bypass,
                    replica_groups=groups,
                    ins=[kin_v.opt()],
                    outs=[kout_v.opt()],
                )

                # --- q projection + norm + rope (overlaps the collectives)
                y_sb = make_y()
                q_sb = kvpool.tile([P, HH, TOK], f8, tag="kq", name="q_sb")
                proj_heads(Wq_t, y_sb, q_sb, mq, cosq_sb, sinq_sb, "wq")

                # --- unpack gathered k/v (emitted after q DMAs so the sync
                # queue doesn't block q-side work on the collective)
                k_all = kvpool.tile(
                    [P, HH, NR, TOK], f8, tag="k_all", name="k_all"
                )
                v_all = kvpool.tile(
                    [P, NR, TC, H, VW], f8, tag="v_all", name="v_all"
                )
                for r in range(NR):
                    nc.sync.dma_start(
                        k_all[:, :, r],
                        kout_k[r].rearrange(
                            "(p h t) -> p h t", p=P, h=HH, t=TOK
                        ),
                    )
                for r in range(NR):
                    nc.sync.dma_start(
                        v_all[:, r],
                        kout_v[r].rearrange(
                            "(p a b c) -> p a b c", p=P, a=TC, b=H, c=VW
                        ),
                    )

                # --- attention: per head-pair, all 16 score/exp chunks then
                # the 32 av matmuls (prob run-ahead absorbs collective
                # latency); denominators ride in row 64 of px (ones col of v)
                xT = kvpool.tile([P, HH, TOK], f8, tag="xT", name="xT")
                # software pipeline: head-pair hh's scores+exp interleave with
                # head-pair hh-1's av matmuls so neither PE nor ACT drains
                prev = None  # (probs, px, hh-1)
                for hh in range(HH + 1):
                    probs = []
                    px = None
                    if hh < HH:
                        px = [
                            ps_x.tile(
                                [VW, TOK], f32, tag=f"px{i}", name=f"px{i}"
                            )
                            for i in range(2)
                        ]
                    prob = None
                    for kc in range(H):  # 16 k-chunks of 128 tokens
                        r, tcl = kc // TC, kc % TC
                        if hh < HH:
                            ps = ps_s.tile(
                                [P, 2 * TOK], f32, tag="ps", name="ps"
                            )
                            for i in range(2):
                                off = i * D
                                nc.tensor.matmul(
                                    ps[:, i * TOK : (i + 1) * TOK],
                                    k_all[
                                        off : off + D,
                                        hh,
                                        r,
                                        tcl * P : (tcl + 1) * P,
                                    ],
                                    q_sb[off : off + D, hh],
                                    start=True,
                                    stop=True,
                                )
                            if kc % 2 == 0:
                                prob = probp.tile(
                                    [P, 2, 2 * TOK], f8, tag="prob",
                                    name="prob",
                                )
                            # exp(s/sqrt(d) - 2): the -2 cancels against the
                            # denominator and keeps prob magnitudes in fp8
                            # range
                            nc.scalar.activation(
                                prob[:, kc % 2, :], ps[:], AF.Exp,
                                scale=1.0 / math.sqrt(D), bias=negb_sb[:],
                            )
                            if kc % 2 == 1:
                                probs.append((prob, r, tcl // 2))
                        if prev is not None and kc % 2 == 1:
                            pprobs, ppx, phh = prev
                            pprob, r0, tp0 = pprobs[kc // 2]
                            for i in range(2):
                                h = phh * 2 + i
                                nc.tensor.matmul(
                                    ppx[i][:],
                                    v_all[:, r0, 2 * tp0 : 2 * tp0 + 2, h, :],
                                    pprob[:, :, i * TOK : (i + 1) * TOK],
                                    perf_mode=DR,
                                    start=(kc == 1),
                                    stop=(kc == H - 1),
                                )
                    if prev is not None:
                        # --- head pair phh epilogue: copy out the (still
                        # unnormalized) numerators and the denominator row
                        # first so the av psum frees fast, then broadcast the
                        # raw denominators via a tiny DRAM bounce and divide
                        # full-width off the critical path
                        pprobs, ppx, phh = prev
                        xraw = rpool.tile(
                            [P, TOK], bf16, tag="xraw", name="xraw"
                        )
                        drec = drpool.tile(
                            [VW, 2, TOK], bf16, tag="drec", name="drec"
                        )
                        for i in range(2):
                            nc.vector.tensor_copy(
                                xraw[i * D : (i + 1) * D], ppx[i][0:D]
                            )
                            nc.vector.tensor_copy(
                                drec[D : D + 1, i], ppx[i][D : D + 1]
                            )
                        db = dbp.tile([2 * TOK], bf16, tag="db")
                        nc.sync.dma_start(
                            db[:].rearrange("(o a t) -> o a t", o=1, a=2),
                            drec[D : D + 1],
                        )
                        rb = rpool.tile([P, TOK], bf16, tag="rb", name="rb")
                        for i in range(2):
                            src = bass.AP(
                                tensor=db.tensor,
                                offset=db.offset + i * TOK,
                                ap=[[0, D], [1, TOK]],
                            )
                            nc.sync.dma_start(rb[i * D : (i + 1) * D], src)
                        rbr = rpool.tile([P, TOK], bf16, tag="rbr", name="rbr")
                        nc.vector.reciprocal(rbr[:], rb[:])
                        nc.vector.tensor_mul(xT[:, phh], xraw[:], rbr[:])
                    prev = (probs, px, hh) if hh < HH else None

                # --- Wo projection, accumulate into resid (undoing the
                # SV*SW fp8 weight scales)
                for g in range(2):
                    wo = wpool.tile([P, 4, KO, P], f8, tag="w1m", name="wo")
                    nc.sync.dma_start(
                        wo[:],
                        Wo_t[g * 4 : (g + 1) * 4].rearrange(
                            "g p ko m -> p g ko m"
                        ),
                    )
                    for j in range(4):
                        oc = g * 4 + j
                        po = pp.tile([P, TOK], f32, tag="pp", name="po")
                        for c2 in range(KO // 2):
                            nc.tensor.matmul(
                                po[:],
                                wo[:, j, 2 * c2 : 2 * c2 + 2, :],
                                xT[:, 2 * c2 : 2 * c2 + 2, :],
                                perf_mode=DR,
                                start=(c2 == 0),
                                stop=(c2 == KO // 2 - 1),
                            )
                        nc.vector.scalar_tensor_tensor(
                            resid[:, oc], po[:], 1.0 / (SV * SW),
                            resid[:, oc],
                            op0=mybir.AluOpType.mult, op1=mybir.AluOpType.add,
                        )

            # ================= cross-attention =================
            yT = sing.tile([P, KO, TOK], f8, name="yT")

            def make_y_ca():
                rmsnorm_feat(resid, yT)
                return yT

            attention_block(
                srcT_sb, caWq, caWk, caWv, caWo,
                masks_sb["mq_ca"], masks_sb["mk_ca"], coskca_sb, sinkca_sb,
                make_y_ca, "ca",
            )

            # ================= self-attention =================
            rmsnorm_feat(resid, yT)
            attention_block(
                yT, saWq, saWk, saWv, saWo,
                masks_sb["mq_sa"], masks_sb["mk_sa"], cosq_sb, sinq_sb,
                lambda: yT, "sa",
            )

            # ================= FFN =================
            rmsnorm_feat(resid, yT)
            hT = kvpool.tile([P, HC, TOK], f8, tag="k_all", name="hT")
            for g in range(8):  # stream W1/W3 in eighths
                w1 = wpool.tile([P, 4, KO, P], f8, tag="w1m", name="w1")
                nc.sync.dma_start(
                    w1[:],
                    W1i[g * 4 : (g + 1) * 4].rearrange("g p ko m -> p g ko m"),
                )
                w3 = wpool.tile([P, 4, KO, P], f8, tag="w1m", name="w3")
                nc.sync.dma_start(
                    w3[:],
                    W3i[g * 4 : (g + 1) * 4].rearrange("g p ko m -> p g ko m"),
                )
                for j in range(4):
                    hc = g * 4 + j
                    p1 = pp.tile([P, TOK], f32, tag="pp", name="p1")
                    for c2 in range(KO // 2):
                        nc.tensor.matmul(
                            p1[:],
                            w1[:, j, 2 * c2 : 2 * c2 + 2, :],
                            yT[:, 2 * c2 : 2 * c2 + 2, :],
                            perf_mode=DR,
                            start=(c2 == 0), stop=(c2 == KO // 2 - 1),
                        )
                    p3 = ps_s.tile([P, TOK], f32, tag="ps", name="p3")
                    for c2 in range(KO // 2):
                        nc.tensor.matmul(
                            p3[:],
                            w3[:, j, 2 * c2 : 2 * c2 + 2, :],
                            yT[:, 2 * c2 : 2 * c2 + 2, :],
                            perf_mode=DR,
                            start=(c2 == 0), stop=(c2 == KO // 2 - 1),
                        )
                    s1 = stat.tile([P, TOK], f32, tag="raw", name="s1")
                    nc.scalar.activation(s1[:], p1[:], AF.Silu, scale=1.0 / SW)
                    # hT = silu(x1) * x3 * SH, undoing W3's SW scale
                    nc.vector.scalar_tensor_tensor(
                        hT[:, hc], p3[:], SH / SW, s1[:],
                        op0=mybir.AluOpType.mult, op1=mybir.AluOpType.mult,
                    )
            for oc in range(KO):
                w2 = w2pool.tile([P, HC, P], f8, tag="w2", name="w2")
                for c4 in range(4):
                    nc.sync.dma_start(
                        w2[:, c4 * 8 : (c4 + 1) * 8],
                        W2i[oc, :, c4 * 8 : (c4 + 1) * 8],
                    )
                po = pp.tile([P, TOK], f32, tag="pp", name="po2")
                for h2 in range(HC // 2):
                    nc.tensor.matmul(
                        po[:],
                        w2[:, 2 * h2 : 2 * h2 + 2, :],
                        hT[:, 2 * h2 : 2 * h2 + 2, :],
                        perf_mode=DR,
                        start=(h2 == 0), stop=(h2 == HC // 2 - 1),
                    )
                nc.vector.scalar_tensor_tensor(
                    resid[:, oc], po[:], 1.0 / (SH * SW), resid[:, oc],
                    op0=mybir.AluOpType.mult, op1=mybir.AluOpType.add,
                )

            nc.sync.dma_start(outT[:], resid[:])

    _split_multiwait(nc)
    return nc


def _prep_inputs(inputs):
    """Full problem inputs -> list of 8 per-core in_maps."""
    tgt = np.asarray(inputs["tgt"], np.float32)
    src = np.asarray(inputs["src"], np.float32)
    tgt_pos = np.asarray(inputs["tgt_pos"], np.int32)
    src_pos = np.asarray(inputs["src_pos"], np.int32)

    pre_ca_w = np.asarray(inputs["pre_ca_w"], np.float32)
    pre_sa_w = np.asarray(inputs["pre_sa_w"], np.float32)
    pre_ffn_w = np.asarray(inputs["pre_ffn_w"], np.float32)

    def fold(Wname, w):
        return np.asarray(inputs[Wname], np.float32) * w[:, None]

    ca_Wq = fold("ca_Wq", pre_ca_w)
    ca_Wkv = np.asarray(inputs["ca_Wkv"], np.float32)
    ca_Wk, ca_Wv = ca_Wkv[:, :DIM], ca_Wkv[:, DIM:]
    ca_Wo = np.asarray(inputs["ca_Wo"], np.float32)
    sa_Wq = fold("sa_Wq", pre_sa_w)
    sa_Wkv = fold("sa_Wkv", pre_sa_w)
    sa_Wk, sa_Wv = sa_Wkv[:, :DIM], sa_Wkv[:, DIM:]
    sa_Wo = np.asarray(inputs["sa_Wo"], np.float32)
    W1 = fold("W1", pre_ffn_w)
    W3 = fold("W3", pre_ffn_w)
    W2 = np.asarray(inputs["W2"], np.float32)

    shared = {
        "caWq": _lhsT_f8(ca_Wq, SW),
        "caWk": _lhsT_f8(ca_Wk, SW),
        "caWv": _rhs_f8(ca_Wv, SV),
        "caWo": _lhsT_f8(ca_Wo, SW),
        "saWq": _lhsT_f8(sa_Wq, SW),
        "saWk": _lhsT_f8(sa_Wk, SW),
        "saWv": _rhs_f8(sa_Wv, SV),
        "saWo": _lhsT_f8(sa_Wo, SW),
        "W1": _lhsT_f8(W1, SW),
        "W3": _lhsT_f8(W3, SW),
        "W2": _lhsT_f8(W2, SW),
    }

    blk2 = np.zeros((P, 2), BF)
    blk2[:D, 0] = 1
    blk2[D:, 1] = 1
    shared["blk2"] = blk2

    def head_mask(w):  # [2, 128] with per-head norm weight
        m = np.zeros((2, P), np.float32)
        m[0, :D] = w
        m[1, D:] = w
        return m.astype(BF).copy()

    shared["mq_ca"] = head_mask(np.asarray(inputs["ca_qn"], np.float32))
    shared["mk_ca"] = head_mask(np.asarray(inputs["ca_kn"], np.float32))
    shared["mq_sa"] = head_mask(np.asarray(inputs["sa_qn"], np.float32))
    shared["mk_sa"] = head_mask(np.asarray(inputs["sa_kn"], np.float32))

    r64 = np.zeros((D, D), np.float32)
    half = D // 2
    for j in range(half):
        r64[j, j + half] = -1.0  # rot[j] = -x[j+32]
        r64[j + half, j] = 1.0  # rot[j+32] = x[j]
    rt = r64.T  # lhsT (matmul computes lhsT.T @ rhs)
    rotm = np.zeros((P, P), np.float32)
    rotm[:D, :D] = rt
    rotm[D:, D:] = rt
    shared["rotm"] = rotm.astype(BF).copy()

    shared["ones_c"] = np.ones((P, 1), BF)
    shared["ones_r128"] = np.ones((1, P), BF)

    in_maps = []
    for c in range(NCORES):
        s, part = c // NR, c % NR
        rows = slice(part * TOK, (part + 1) * TOK)
        m = dict(shared)
        m["tgtT"] = _featmajor(tgt[s, rows])
        m["srcTb"] = np.clip(_featmajor(src[s, rows]), -240.0, 240.0).astype(F8)
        cq, sq_ = _rope_tables(tgt_pos[s, rows])
        ck, sk = _rope_tables(src_pos[s, rows])
        m["cosq"], m["sinq"] = cq, sq_
        m["coskca"], m["sinkca"] = ck, sk
        in_maps.append(m)
    return in_maps


def _get_nc():
    if "nc" not in _cache:
        _cache["nc"] = _build_bass()
    return _cache["nc"]


def run(inputs, trace=False):
    """Run on 8 cores; returns (full_output, exec_time_ns_or_None)."""
    if trace:
        _install_ntff_hook()
    from concourse.bass_utils import run_bass_kernel_spmd

    in_maps = _prep_inputs(inputs)
    nc = _get_nc()
    res = run_bass_kernel_spmd(
        nc, in_maps, core_ids=list(range(NCORES)), trace=trace
    )
    out = np.empty((B, N, DIM), np.float32)
    for c in range(NCORES):
        s, part = c // NR, c % NR
        arr = np.asarray(res.results[c]["outT"])  # [128, 8, TOK]
        rows = slice(part * TOK, (part + 1) * TOK)
        out[s, rows] = np.transpose(arr, (2, 1, 0)).reshape(TOK, DIM)
    return out, res.exec_time_ns


def kernel(**inputs):
    out, _ = run(inputs, trace=False)
    return out
